# revision 1
# baseline (speedup 1.0000x reference)
"""DiffusionTransformer (AF3-style) Trainium2 kernel, 8-core SPMD.

Sharding: sequence-parallel over rows (queries). Each core owns R=128 rows of
a / z / beta. Per block, k/v are computed on local rows and AllGathered
(bf16, ~384KB per rank). zbeta (pair bias LN(z)@wpb + beta, all 4 blocks) is
precomputed once into per-core DRAM in [bh, i, j] layout.
"""
import numpy as np
import ml_dtypes
from contextlib import ExitStack

import concourse.bass as bass
import concourse.mybir as mybir
import concourse.tile as tile
from concourse import bacc
from concourse.bass_utils import run_bass_kernel_spmd
from concourse.masks import make_identity

NB, H, S, CA, CS, CZ = 4, 16, 1024, 768, 384, 128
D = CA // H            # 48
NCORE = 8
R = S // NCORE         # 128 rows per core
NHID = 2 * CA          # 1536
EPS = 1e-5
BF = mybir.dt.bfloat16
F32 = mybir.dt.float32
AX = mybir.AxisListType
OP = mybir.AluOpType
ACTF = mybir.ActivationFunctionType
JT = S // 128          # 8 j-tiles
KT_A = CA // 128       # 6 k-tiles over c_a
KT_S = CS // 128       # 3 k-tiles over c_s
KT_H = NHID // 128     # 12 k-tiles over n_hidden
KV_K = D * H * R       # 98304 flat elems of kT part
KV_TOT = KV_K + R * CA  # + v part


def _headsplit_ranges():
    """Split [0,768) hd-range at both 128-tile and 48-head boundaries.

    Returns list of (kt, p0, p1, h, d0) with kt*128+p0 == h*48+d0.
    """
    out = []
    bounds = sorted(set([x * 128 for x in range(KT_A + 1)] +
                        [h * D for h in range(H + 1)]))
    for lo, hi in zip(bounds[:-1], bounds[1:]):
        kt, p0 = lo // 128, lo % 128
        h, d0 = lo // D, lo % D
        out.append((kt, p0, hi - lo, h, d0))
    return out


HS = _headsplit_ranges()


def build_program(bias_pb):
    nc = bacc.Bacc("TRN2", target_bir_lowering=False, debug=False,
                   num_devices=NCORE)

    # ---------------- I/O ----------------
    a_in = nc.dram_tensor("a_in", [R, CA], F32, kind="ExternalInput")
    s_in = nc.dram_tensor("s_in", [R, CS], F32, kind="ExternalInput")
    z_in = nc.dram_tensor("z_in", [R, S, CZ], BF, kind="ExternalInput")
    betaT_in = nc.dram_tensor("betaT_in", [H, R, S], BF, kind="ExternalInput")
    wz_in = nc.dram_tensor("wz_in", [CZ, NB * H], BF, kind="ExternalInput")
    sel_in = nc.dram_tensor("sel_in", [H, NB * H], BF, kind="ExternalInput")
    wsn_in = nc.dram_tensor("wsn_in", [CS, NB, 4 * CA], BF, kind="ExternalInput")
    wsr_in = nc.dram_tensor("wsr_in", [CS + 1, NB, 2 * CA], BF, kind="ExternalInput")
    wkv_in = nc.dram_tensor("wkv_in", [CA, NB, 2 * CA], BF, kind="ExternalInput")
    wqg_in = nc.dram_tensor("wqg_in", [CA, NB, 2 * CA], BF, kind="ExternalInput")
    bq_in = nc.dram_tensor("bq_in", [NB, CA], F32, kind="ExternalInput")
    wsw_in = nc.dram_tensor("wsw_in", [CA, NB, NHID], BF, kind="ExternalInput")
    wg2_in = nc.dram_tensor("wg2_in", [CA, NB, NHID], BF, kind="ExternalInput")
    wo_in = nc.dram_tensor("wo_in", [CA, NB, CA], BF, kind="ExternalInput")
    wout_in = nc.dram_tensor("wout_in", [NHID, NB, CA], BF, kind="ExternalInput")
    a_out = nc.dram_tensor("a_out", [R, CA], F32, kind="ExternalOutput")

    with tile.TileContext(nc) as tc, ExitStack() as ctx:
        const = ctx.enter_context(tc.tile_pool(name="const", bufs=1))
        ident = const.tile([128, 128], BF)
        make_identity(nc, ident)
        wz_sb = const.tile([CZ, NB * H], BF)
        nc.sync.dma_start(out=wz_sb, in_=wz_in[:])
        sel_sb = const.tile([H, NB * H], BF)
        nc.sync.dma_start(out=sel_sb, in_=sel_in[:])
        biases = const.tile([128, 1 + NB * H], F32)
        nc.vector.memset(biases[:, 0:1], EPS)
        for _bh in range(NB * H):
            nc.vector.memset(biases[:, 1 + _bh:2 + _bh], float(bias_pb[_bh]))

        pers = ctx.enter_context(tc.tile_pool(name="pers", bufs=1))
        a_sb = pers.tile([R, CA], F32)
        nc.sync.dma_start(out=a_sb, in_=a_in[:])

        # internal DRAM scratch
        dram = ctx.enter_context(tc.tile_pool(name="dram", bufs=1, space="DRAM"))
        zbeta_dr = dram.tile([NB * H, R, S], BF)

        # =========== s preprocessing (once) ===========
        with tc.tile_pool(name="sprep", bufs=1) as sp:
            s_sb = sp.tile([R, CS], F32)
            nc.sync.dma_start(out=s_sb, in_=s_in[:])
            stats = sp.tile([R, 1, 6], F32)
            mv = sp.tile([R, 2], F32)
            nc.vector.bn_stats(out=stats[:, 0, :], in_=s_sb[:])
            nc.vector.bn_aggr(out=mv, in_=stats)
            rstd = sp.tile([R, 1], F32)
            nc.scalar.activation(out=rstd, in_=mv[:, 1:2], func=ACTF.Sqrt,
                                 bias=biases[:, 0:1], scale=1.0)
            nc.vector.reciprocal(out=rstd, in_=rstd)
            s_n = sp.tile([R, CS], BF)
            nc.vector.tensor_scalar(s_n[:], s_sb[:], mv[:, 0:1], rstd[:, 0:1],
                                    OP.subtract, OP.mult)
            s_b16 = sp.tile([R, CS], BF)
            nc.vector.tensor_copy(s_b16[:], s_sb[:])

            # transposed copies (persistent for all blocks)
            s_nT = pers.tile([128, KT_S, 128], BF)
            sT = pers.tile([128, KT_S, 128], BF)
            ones_row = pers.tile([1, 128], BF)
            nc.vector.memset(ones_row, 1.0)
            with tc.tile_pool(name="tp_ps", bufs=2, space="PSUM") as tps:
                for kt in range(KT_S):
                    pt = tps.tile([128, 128], BF, tag="t")
                    nc.tensor.transpose(pt[:], s_n[:, bass.ts(kt, 128)], ident[:])
                    nc.scalar.copy(out=s_nT[:, kt, :], in_=pt[:])
                    pt2 = tps.tile([128, 128], BF, tag="t")
                    nc.tensor.transpose(pt2[:], s_b16[:, bass.ts(kt, 128)], ident[:])
                    nc.scalar.copy(out=sT[:, kt, :], in_=pt2[:])

            # per-block s-derived tensors: sg/sb for attn+tr, gates attn/tr
            sgsb = pers.tile([R, NB, 4 * CA], BF)   # wg_a|wb_a|wg_t|wb_t
            gts = pers.tile([R, NB, 2 * CA], BF)    # gate_attn|gate_tr
            with tc.tile_pool(name="sw", bufs=2) as swp, \
                 tc.tile_pool(name="sps", bufs=3, space="PSUM") as sps:
                for b in range(NB):
                    wsn_sb = swp.tile([128, KT_S, 4 * CA], BF, tag="wsn")
                    nc.sync.dma_start(
                        out=wsn_sb,
                        in_=wsn_in[:].rearrange("(kt p) b m -> p kt b m", p=128)[:, :, b, :])
                    for m in range(6):  # 3072 / 512
                        ps = sps.tile([R, 512], F32, tag="ps")
                        for kt in range(KT_S):
                            nc.tensor.matmul(ps[:], s_nT[:, kt, :],
                                             wsn_sb[:, kt, bass.ts(m, 512)],
                                             start=(kt == 0), stop=(kt == KT_S - 1))
                        # cols [m*512,(m+1)*512) of [wg_a(768)|wb_a|wg_t|wb_t]
                        for lo_, hi_ in [(m * 512, m * 512 + 256), (m * 512 + 256, (m + 1) * 512)]:
                            mat = lo_ // CA  # 0..3
                            f = ACTF.Sigmoid if mat in (0, 2) else ACTF.Copy
                            nc.scalar.activation(
                                out=sgsb[:, b, lo_:hi_], in_=ps[:, lo_ - m * 512:hi_ - m * 512],
                                func=f, bias=0.0 if f == ACTF.Copy else 0.0, scale=1.0)
                    wsr_sb = swp.tile([128, KT_S, 2 * CA], BF, tag="wsr")
                    nc.sync.dma_start(
                        out=wsr_sb,
                        in_=wsr_in[:CS].rearrange("(kt p) b m -> p kt b m", p=128)[:, :, b, :])
                    wsr_last = swp.tile([1, 2 * CA], BF, tag="wsrl")
                    nc.sync.dma_start(out=wsr_last, in_=wsr_in[CS:CS + 1, b, :])
                    for m in range(3):  # 1536 / 512
                        ps = sps.tile([R, 512], F32, tag="ps")
                        for kt in range(KT_S):
                            nc.tensor.matmul(ps[:], sT[:, kt, :],
                                             wsr_sb[:, kt, bass.ts(m, 512)],
                                             start=(kt == 0), stop=False)
                        nc.tensor.matmul(ps[:], ones_row[:],
                                         wsr_last[:, bass.ts(m, 512)],
                                         start=False, stop=True)
                        nc.scalar.activation(out=gts[:, b, bass.ts(m, 512)],
                                             in_=ps[:], func=ACTF.Sigmoid,
                                             bias=0.0, scale=1.0)

        # =========== z preprocessing (once) ===========
        with tc.tile_pool(name="zslab", bufs=4) as zsl, \
             tc.tile_pool(name="zsm", bufs=4) as zsm, \
             tc.tile_pool(name="ztp", bufs=3, space="PSUM") as ztp, \
             tc.tile_pool(name="zbp", bufs=2, space="PSUM") as zbp:
            z_r = z_in[:].rearrange("i (jt jp) c -> i jp jt c", jp=128)
            for i in range(R):
                zt = zsl.tile([128, JT, CZ], BF, tag="z")
                for _q in range(4):
                    nc.sync.dma_start(out=zt[:, 2 * _q:2 * _q + 2, :],
                                      in_=z_r[i][:, 2 * _q:2 * _q + 2, :])
                bsel = zsl.tile([H, S], BF, tag="bsel")
                nc.sync.dma_start(out=bsel, in_=betaT_in[:, i, :])
                st8 = zsm.tile([128, JT, 6], F32, tag="st")
                for jt in range(JT):
                    nc.vector.bn_stats(out=st8[:, jt, :], in_=zt[:, jt, :])
                # pooled even/odd moments, vectorized over all 8 j-tiles:
                # mean = (m_e+m_o)/2; var = (64v_e+64v_o)/128 + ((m_e-m_o)/2)^2
                mrow = zsm.tile([128, JT], F32, tag="mrow")
                nc.vector.tensor_tensor(mrow[:], st8[:, :, 1], st8[:, :, 4], OP.add)
                nc.vector.tensor_scalar(mrow[:], mrow[:], 0.5, None, OP.mult)
                dm = zsm.tile([128, JT], F32, tag="dm")
                nc.vector.tensor_tensor(dm[:], st8[:, :, 1], st8[:, :, 4],
                                        OP.subtract)
                nc.vector.tensor_tensor(dm[:], dm[:], dm[:], OP.mult)
                nc.vector.tensor_scalar(dm[:], dm[:], 0.25, None, OP.mult)
                var = zsm.tile([128, JT], F32, tag="var")
                nc.vector.tensor_tensor(var[:], st8[:, :, 2], st8[:, :, 5], OP.add)
                nc.vector.tensor_scalar(var[:], var[:], 1.0 / CZ, None, OP.mult)
                nc.vector.tensor_tensor(var[:], var[:], dm[:], OP.add)
                rst = zsm.tile([128, JT], F32, tag="rst")
                nc.scalar.activation(out=rst, in_=var[:], func=ACTF.Sqrt,
                                     bias=biases[:, 0:1], scale=1.0)
                nc.vector.reciprocal(out=rst, in_=rst)
                zh = zsm.tile([128, JT, CZ], BF, tag="zh")
                for jt in range(JT):
                    nc.vector.tensor_scalar(zh[:, jt, :], zt[:, jt, :],
                                            mrow[:, jt:jt + 1], rst[:, jt:jt + 1],
                                            OP.subtract, OP.mult)
                zhT = zsm.tile([128, JT, 128], BF, tag="zhT")
                for g in range(2):  # transpose 8 tiles, copy in 2 batches
                    pt = ztp.tile([128, 4, 128], BF, tag="t")
                    for q in range(4):
                        jt = g * 4 + q
                        nc.tensor.transpose(pt[:, q, :], zh[:, jt, :], ident[:])
                    nc.scalar.copy(out=zhT[:, g * 4:(g + 1) * 4, :], in_=pt[:])
                zb = zbp.tile([NB * H, S], F32, tag="zb")
                for jc in range(2):
                    nc.tensor.matmul(zb[:, bass.ts(jc, 512)], wz_sb[:],
                                     zhT[:].rearrange("p jt j -> p (jt j)")[:, bass.ts(jc, 512)],
                                     start=True, stop=False)
                    nc.tensor.matmul(zb[:, bass.ts(jc, 512)], sel_sb[:],
                                     bsel[:, bass.ts(jc, 512)],
                                     start=False, stop=True)
                zbs = zsm.tile([NB * H, S], BF, tag="zbs")
                nc.scalar.copy(out=zbs, in_=zb[:])
                nc.sync.dma_start(out=zbeta_dr[:, i, :], in_=zbs)

        # =========== block loop ===========
        wpool = ctx.enter_context(tc.tile_pool(name="wpool", bufs=2))
        wop = ctx.enter_context(tc.tile_pool(name="wop", bufs=1))
        blk = ctx.enter_context(tc.tile_pool(name="blk", bufs=1))
        kvg = ctx.enter_context(tc.tile_pool(name="kvg", bufs=1))
        att = ctx.enter_context(tc.tile_pool(name="att", bufs=3))
        dramc = ctx.enter_context(tc.tile_pool(name="dramc", bufs=2, space="DRAM"))

        for b in range(NB):
            # ---- ada_ln(a) shared stats ----
            with tc.tile_pool(name="lnp", bufs=1) as lnp, \
                 tc.tile_pool(name="ps_ln", bufs=2, space="PSUM") as pln:
                st3 = lnp.tile([R, 3, 6], F32)
                for g_ in range(3):
                    nc.vector.bn_stats(out=st3[:, g_, :],
                                       in_=a_sb[:, bass.ts(g_, 256)])
                mv = lnp.tile([R, 2], F32)
                nc.vector.bn_aggr(out=mv, in_=st3)
                rstd = lnp.tile([R, 1], F32)
                nc.scalar.activation(out=rstd, in_=mv[:, 1:2], func=ACTF.Sqrt,
                                     bias=biases[:, 0:1], scale=1.0)
                nc.vector.reciprocal(out=rstd, in_=rstd)
                xhat = lnp.tile([R, CA], BF)
                nc.vector.tensor_scalar(xhat[:], a_sb[:], mv[:, 0:1], rstd[:, 0:1],
                                        OP.subtract, OP.mult)
                ah = blk.tile([R, CA], BF, tag="ah")
                nc.vector.tensor_tensor(ah[:], xhat[:], sgsb[:, b, 0:CA], OP.mult)
                nc.vector.tensor_tensor(ah[:], ah[:], sgsb[:, b, CA:2 * CA], OP.add)
                th = blk.tile([R, CA], BF, tag="th")
                nc.vector.tensor_tensor(th[:], xhat[:], sgsb[:, b, 2 * CA:3 * CA], OP.mult)
                nc.vector.tensor_tensor(th[:], th[:], sgsb[:, b, 3 * CA:4 * CA], OP.add)
                ahT = blk.tile([128, KT_A, 128], BF, tag="ahT")
                for kt in range(KT_A):
                    pt = pln.tile([128, 128], BF, tag="t")
                    nc.tensor.transpose(pt[:], ah[:, bass.ts(kt, 128)], ident[:])
                    nc.scalar.copy(out=ahT[:, kt, :], in_=pt[:])
                thT = blk.tile([128, KT_A, 128], BF, tag="thT")
                for kt in range(KT_A):
                    pt = pln.tile([128, 128], BF, tag="t")
                    nc.tensor.transpose(pt[:], th[:, bass.ts(kt, 128)], ident[:])
                    nc.scalar.copy(out=thT[:, kt, :], in_=pt[:])

            # ---- kv local + gather ----
            kv_inb = dramc.tile([KV_TOT], BF, tag="kvin")
            kv_outb = dramc.tile([NCORE * KV_TOT], BF, tag="kvout",
                                 addr_space="Shared")
            with tc.tile_pool(name="ps_kv", bufs=3, space="PSUM") as pkv:
                wkv_sb = wpool.tile([128, KT_A, 2 * CA], BF, tag="w1536")
                nc.sync.dma_start(
                    out=wkv_sb,
                    in_=wkv_in[:].rearrange("(kt p) b m -> p kt b m", p=128)[:, :, b, :])
                kv_sb = kvg.tile([R, 2 * CA], BF, tag="kv")
                for m in range(3):  # 1536/512
                    ps = pkv.tile([R, 512], F32, tag="ps")
                    for kt in range(KT_A):
                        nc.tensor.matmul(ps[:], ahT[:, kt, :],
                                         wkv_sb[:, kt, bass.ts(m, 512)],
                                         start=(kt == 0), stop=(kt == KT_A - 1))
                    nc.scalar.copy(out=kv_sb[:, bass.ts(m, 512)], in_=ps[:])
                # kT head-tiles
                kT_loc = kvg.tile([D, H, 128], BF, tag="kT")
                for h in range(H):
                    pt = pkv.tile([128, 128], BF, tag="t")
                    nc.tensor.transpose(pt[:D, :], kv_sb[:, h * D:(h + 1) * D], ident[:])
                    nc.vector.tensor_copy(kT_loc[:, h, :], pt[:D, :])
                nc.sync.dma_start(out=kv_inb[0:KV_K].rearrange("(d x) -> d x", d=D),
                                  in_=kT_loc[:].rearrange("d h j -> d (h j)"))
                nc.sync.dma_start(out=kv_inb[KV_K:].rearrange("(j c) -> j c", j=R),
                                  in_=kv_sb[:, CA:])
            nc.gpsimd.collective_compute(
                "AllGather", OP.bypass,
                replica_groups=[list(range(NCORE))],
                ins=[kv_inb[:].opt()], outs=[kv_outb[:].opt()])

            # ---- qT, gT (transposed head layout) ----
            with tc.tile_pool(name="ps_qg", bufs=3, space="PSUM") as pqg:
                wqg_sb = wpool.tile([128, KT_A, 2 * CA], BF, tag="w1536")
                nc.sync.dma_start(
                    out=wqg_sb,
                    in_=wqg_in[:].rearrange("(kt p) b m -> p kt b m", p=128)[:, :, b, :])
                bq_sb = blk.tile([D, H], F32, tag="bq")
                nc.sync.dma_start(
                    out=bq_sb, in_=bq_in[b].rearrange("(h d) -> d h", d=D))
                qT = blk.tile([D, H, 128], BF, tag="qT")
                gT = blk.tile([D, H, 128], BF, tag="gT")
                for h in range(H):
                    ps = pqg.tile([D, 128], F32, tag="ps")
                    for kt in range(KT_A):
                        nc.tensor.matmul(ps[:], wqg_sb[:, kt, h * D:(h + 1) * D],
                                         ahT[:, kt, :],
                                         start=(kt == 0), stop=(kt == KT_A - 1))
                    nc.scalar.activation(out=qT[:, h, :], in_=ps[:],
                                         func=ACTF.Identity,
                                         bias=bq_sb[:, h:h + 1], scale=1.0)
                    ps2 = pqg.tile([D, 128], F32, tag="ps")
                    for kt in range(KT_A):
                        nc.tensor.matmul(ps2[:], wqg_sb[:, kt, CA + h * D:CA + (h + 1) * D],
                                         ahT[:, kt, :],
                                         start=(kt == 0), stop=(kt == KT_A - 1))
                    nc.scalar.activation(out=gT[:, h, :], in_=ps2[:],
                                         func=ACTF.Sigmoid, bias=0.0, scale=1.0)

            # ---- transition: hiddenT = silu(th@wsw)^T * (th@wg2)^T ----
            hidT = blk.tile([128, KT_H, 128], BF, tag="hidT")
            with tc.tile_pool(name="ps_h", bufs=3, space="PSUM") as psh:
                wsw_sb = wpool.tile([128, KT_A, NHID], BF, tag="w1536")
                nc.sync.dma_start(
                    out=wsw_sb,
                    in_=wsw_in[:].rearrange("(kt p) b m -> p kt b m", p=128)[:, :, b, :])
                swT = blk.tile([128, KT_H, 128], BF, tag="swT")
                for mt in range(KT_H):
                    ps = psh.tile([128, 128], F32, tag="ps")
                    for kt in range(KT_A):
                        nc.tensor.matmul(ps[:], wsw_sb[:, kt, bass.ts(mt, 128)],
                                         thT[:, kt, :],
                                         start=(kt == 0), stop=(kt == KT_A - 1))
                    nc.scalar.activation(out=swT[:, mt, :], in_=ps[:],
                                         func=ACTF.Silu, bias=0.0, scale=1.0)
                wg2_sb = wpool.tile([128, KT_A, NHID], BF, tag="w1536")
                nc.sync.dma_start(
                    out=wg2_sb,
                    in_=wg2_in[:].rearrange("(kt p) b m -> p kt b m", p=128)[:, :, b, :])
                for mt in range(KT_H):
                    ps = psh.tile([128, 128], F32, tag="ps")
                    for kt in range(KT_A):
                        nc.tensor.matmul(ps[:], wg2_sb[:, kt, bass.ts(mt, 128)],
                                         thT[:, kt, :],
                                         start=(kt == 0), stop=(kt == KT_A - 1))
                    g2 = att.tile([128, 128], BF, tag="g2")
                    nc.scalar.copy(out=g2, in_=ps[:])
                    nc.vector.tensor_tensor(hidT[:, mt, :], swT[:, mt, :], g2[:],
                                            OP.mult)

            # ---- attention ----
            kv_outr = kv_outb[:].rearrange("(r x) -> r x", r=NCORE)
            kT_src = kv_outr[:, 0:KV_K].rearrange(
                "r (d h j) -> d h r j", d=D, h=H)
            v_full = kvg.tile([128, NCORE, CA], BF, tag="vf")
            v_src = kv_outb[:].rearrange("(r x) -> r x", r=NCORE)[:, KV_K:] \
                .rearrange("r (j c) -> j r c", j=R)
            for _q in range(4):
                nc.sync.dma_start(out=v_full[:, 2 * _q:2 * _q + 2, :],
                                  in_=v_src[:, 2 * _q:2 * _q + 2, :])
            go_T = blk.tile([D, H, 128], BF, tag="goT")
            sums = blk.tile([R, H], F32, tag="sums")
            with tc.tile_pool(name="ps_s", bufs=2, space="PSUM") as pss, \
                 tc.tile_pool(name="ps_t", bufs=2, space="PSUM") as pst, \
                 tc.tile_pool(name="ps_o", bufs=2, space="PSUM") as pso:
                for h in range(H):
                    kT_h = att.tile([D, NCORE, 128], BF, tag="kTh")
                    nc.sync.dma_start(out=kT_h[:, 0:4, :], in_=kT_src[:, h, 0:4, :])
                    nc.sync.dma_start(out=kT_h[:, 4:8, :], in_=kT_src[:, h, 4:8, :])
                    ps_s = pss.tile([R, S], F32, tag="s")
                    for jc in range(2):
                        nc.tensor.matmul(ps_s[:, bass.ts(jc, 512)], qT[:, h, :],
                                         kT_h[:, jc * 4:(jc + 1) * 4, :],
                                         start=True, stop=True)
                    zb_t = att.tile([R, S], BF, tag="zbt")
                    nc.sync.dma_start(out=zb_t[:, 0:512],
                                      in_=zbeta_dr[b * H + h, :, 0:512])
                    nc.sync.dma_start(out=zb_t[:, 512:1024],
                                      in_=zbeta_dr[b * H + h, :, 512:1024])
                    nc.vector.tensor_tensor(ps_s[:], ps_s[:], zb_t[:], OP.add)
                    attn = att.tile([R, S], BF, tag="attn")
                    bh_ = 1 + b * H + h
                    nc.scalar.activation(out=attn, in_=ps_s[:], func=ACTF.Exp,
                                         bias=biases[:, bh_:bh_ + 1], scale=1.0,
                                         accum_out=sums[:, h:h + 1])
                    rec = att.tile([R, 1], F32, tag="rec")
                    nc.vector.reciprocal(out=rec, in_=sums[:, h:h + 1])
                    nc.vector.tensor_scalar(attn[:], attn[:], rec[:, 0:1], None,
                                            OP.mult)
                    attnT = att.tile([128, JT, 128], BF, tag="attnT")
                    for g in range(2):
                        pt = pst.tile([128, 4, 128], BF, tag="t")
                        for q in range(4):
                            jt = g * 4 + q
                            nc.tensor.transpose(pt[:, q, :],
                                                attn[:, bass.ts(jt, 128)], ident[:])
                        nc.vector.tensor_copy(attnT[:, g * 4:(g + 1) * 4, :], pt[:])
                    ps_o = pso.tile([128, 128], F32, tag="o")
                    for jt in range(JT):
                        nc.tensor.matmul(ps_o[:D, :], v_full[:, jt, h * D:(h + 1) * D],
                                         attnT[:, jt, :],
                                         start=(jt == 0), stop=(jt == JT - 1))
                    nc.vector.tensor_tensor(go_T[:, h, :], ps_o[:D, :],
                                            gT[:, h, :], OP.mult)

            # ---- att_out = (g*o) @ wo ; b_attn = gate_attn * att_out ----
            b_attn = blk.tile([R, CA], F32, tag="batt")
            with tc.tile_pool(name="ps_wo", bufs=2, space="PSUM") as pwo:
                wo_sb = wop.tile([D, H, CA], BF, tag="wo")
                nc.sync.dma_start(
                    out=wo_sb,
                    in_=wo_in[:].rearrange("(h d) b m -> d h b m", d=D)[:, :, b, :])
                for m in range(2):
                    n0, n1 = (0, 512) if m == 0 else (512, 768)
                    ps = pwo.tile([R, 512], F32, tag="ps")
                    for h in range(H):
                        nc.tensor.matmul(ps[:, 0:n1 - n0], go_T[:, h, :],
                                         wo_sb[:, h, n0:n1],
                                         start=(h == 0), stop=(h == H - 1))
                    nc.vector.tensor_tensor(b_attn[:, n0:n1], ps[:, 0:n1 - n0],
                                            gts[:, b, n0:n1], OP.mult)

            # ---- tr = gate_tr * (hidden @ w_out); a = b_attn + tr ----
            with tc.tile_pool(name="ps_tr", bufs=2, space="PSUM") as ptr:
                wout_sb = wop.tile([128, KT_H, CA], BF, tag="wout")
                nc.sync.dma_start(
                    out=wout_sb,
                    in_=wout_in[:].rearrange("(kt p) b m -> p kt b m", p=128)[:, :, b, :])
                for m in range(2):
                    n0, n1 = (0, 512) if m == 0 else (512, 768)
                    ps = ptr.tile([R, 512], F32, tag="ps")
                    for kt in range(KT_H):
                        nc.tensor.matmul(ps[:, 0:n1 - n0], hidT[:, kt, :],
                                         wout_sb[:, kt, n0:n1],
                                         start=(kt == 0), stop=(kt == KT_H - 1))
                    tr = att.tile([R, 512], F32, tag="tr")
                    nc.vector.tensor_tensor(tr[:, 0:n1 - n0], ps[:, 0:n1 - n0],
                                            gts[:, b, CA + n0:CA + n1], OP.mult)
                    nc.vector.tensor_tensor(a_sb[:, n0:n1], b_attn[:, n0:n1],
                                            tr[:, 0:n1 - n0], OP.add)

        nc.sync.dma_start(out=a_out[:], in_=a_sb[:])

    nc.finalize()
    return nc


def _prep_inputs(a, s, z, beta, ln_s_w_attn, wg_attn, wb_attn, wq, bq, wk, wv,
                 ln_z_w, ln_z_b, wpb, wgate, wo, wsg_attn, bsg_attn,
                 ln_s_w_tr, wg_tr, wb_tr, w_swish, w_gate2, wsg_tr, bsg_tr, w_out):
    bf = ml_dtypes.bfloat16
    f32 = np.float32
    scale = 1.0 / np.sqrt(np.float32(D))

    # folded weights (shared across cores)
    wz = np.concatenate([ln_z_w[i][:, None] * wpb[i] for i in range(NB)],
                        axis=1).astype(bf)                       # [CZ, NB*H]
    bias_pb = np.concatenate([ln_z_b[i] @ wpb[i] for i in range(NB)])  # [NB*H]
    sel = np.tile(np.eye(H, dtype=np.float32), (1, NB)).astype(bf)  # [H, NB*H]
    wsn = np.stack([np.concatenate(
        [ln_s_w_attn[i][:, None] * wg_attn[i], ln_s_w_attn[i][:, None] * wb_attn[i],
         ln_s_w_tr[i][:, None] * wg_tr[i], ln_s_w_tr[i][:, None] * wb_tr[i]],
        axis=1) for i in range(NB)], axis=1).astype(bf)          # [CS, NB, 4CA]
    wsr = np.stack([np.concatenate(
        [np.concatenate([wsg_attn[i], bsg_attn[i][None, :]], 0),
         np.concatenate([wsg_tr[i], bsg_tr[i][None, :]], 0)], axis=1)
        for i in range(NB)], axis=1).astype(bf)                  # [CS+1, NB, 2CA]
    wkv = np.stack([np.concatenate([wk[i], wv[i]], 1) for i in range(NB)],
                   axis=1).astype(bf)                            # [CA, NB, 2CA]
    wqg = np.stack([np.concatenate([wq[i] * scale, wgate[i]], 1)
                    for i in range(NB)], axis=1).astype(bf)
    bqe = (bq * scale).astype(f32)                               # [NB, CA]
    wsw = np.stack([w_swish[i] for i in range(NB)], axis=1).astype(bf)
    wg2 = np.stack([w_gate2[i] for i in range(NB)], axis=1).astype(bf)
    wob = np.stack([wo[i] for i in range(NB)], axis=1).astype(bf)
    wout = np.stack([w_out[i] for i in range(NB)], axis=1).astype(bf)

    shared = dict(wz_in=np.ascontiguousarray(wz),
                  sel_in=np.ascontiguousarray(sel),
                  wsn_in=np.ascontiguousarray(wsn),
                  wsr_in=np.ascontiguousarray(wsr),
                  wkv_in=np.ascontiguousarray(wkv),
                  wqg_in=np.ascontiguousarray(wqg),
                  bq_in=np.ascontiguousarray(bqe),
                  wsw_in=np.ascontiguousarray(wsw),
                  wg2_in=np.ascontiguousarray(wg2),
                  wo_in=np.ascontiguousarray(wob),
                  wout_in=np.ascontiguousarray(wout))

    a2 = a.reshape(S, CA).astype(f32)
    s2 = s.reshape(S, CS).astype(f32)
    z2 = z.reshape(S, S, CZ).astype(bf)
    betaT = np.ascontiguousarray(
        beta.reshape(S, S, H).transpose(2, 0, 1)).astype(bf)     # [H, S, S]

    in_maps = []
    for c in range(NCORE):
        rows = slice(c * R, (c + 1) * R)
        m = dict(shared)
        m["a_in"] = np.ascontiguousarray(a2[rows])
        m["s_in"] = np.ascontiguousarray(s2[rows])
        m["z_in"] = np.ascontiguousarray(z2[rows])
        m["betaT_in"] = np.ascontiguousarray(betaT[:, rows, :])
        in_maps.append(m)
    return in_maps, [float(x) for x in bias_pb]


_CACHE = {}


def kernel(**inputs):
    inputs = {k: np.asarray(v) for k, v in inputs.items()}
    in_maps, bias_pb = _prep_inputs(**inputs)
    key = tuple(bias_pb)
    if key not in _CACHE:
        _CACHE.clear()
        _CACHE[key] = build_program(bias_pb)
    nc = _CACHE[key]
    res = run_bass_kernel_spmd(nc, in_maps, core_ids=list(range(NCORE)),
                               trace=False)
    out = np.concatenate([res.results[c]["a_out"] for c in range(NCORE)], axis=0)
    return out.reshape(1, S, CA).astype(np.float32)


if __name__ == "__main__":
    import reference
    ins = {k: np.asarray(v) for k, v in reference.setup_inputs().items()}
    exp = np.asarray(reference.reference(**ins))
    act = kernel(**ins)
    err = np.abs(act - exp).max() / (np.abs(exp).max() + 1e-9)
    print("rel err:", err)



# revision 35
# speedup vs baseline: 1.3541x; 1.3541x over previous
"""DiffusionTransformer (AF3-style) Trainium2 kernel, 8-core SPMD.

Sharding: sequence-parallel over rows (queries). Each core owns R=128 rows of
a / z / beta. Per block, k/v are computed on local rows and AllGathered
(bf16, ~384KB per rank). zbeta (pair bias LN(z)@wpb + beta, all 4 blocks) is
precomputed once into per-core DRAM in [bh, i, j] layout.
"""
import numpy as np
import ml_dtypes
from contextlib import ExitStack

import concourse.bass as bass
import concourse.mybir as mybir
import concourse.tile as tile
from concourse import bacc
from concourse.bass_utils import run_bass_kernel_spmd
from concourse.masks import make_identity

NB, H, S, CA, CS, CZ = 4, 16, 1024, 768, 384, 128
D = CA // H            # 48
NCORE = 8
R = S // NCORE         # 128 rows per core
NHID = 2 * CA          # 1536
EPS = 1e-5
BF = mybir.dt.bfloat16
F32 = mybir.dt.float32
AX = mybir.AxisListType
OP = mybir.AluOpType
ACTF = mybir.ActivationFunctionType
JT = S // 128          # 8 j-tiles
KT_A = CA // 128       # 6 k-tiles over c_a
KT_S = CS // 128       # 3 k-tiles over c_s
KT_H = NHID // 128     # 12 k-tiles over n_hidden
KV_K = D * H * R       # 98304 flat elems of kT part
KV_TOT = KV_K + R * CA  # + v part


def _headsplit_ranges():
    """Split [0,768) hd-range at both 128-tile and 48-head boundaries.

    Returns list of (kt, p0, p1, h, d0) with kt*128+p0 == h*48+d0.
    """
    out = []
    bounds = sorted(set([x * 128 for x in range(KT_A + 1)] +
                        [h * D for h in range(H + 1)]))
    for lo, hi in zip(bounds[:-1], bounds[1:]):
        kt, p0 = lo // 128, lo % 128
        h, d0 = lo // D, lo % D
        out.append((kt, p0, hi - lo, h, d0))
    return out


HS = _headsplit_ranges()


def build_program(bias_pb):
    nc = bacc.Bacc("TRN2", target_bir_lowering=False, debug=False,
                   num_devices=NCORE)

    # ---------------- I/O ----------------
    a_in = nc.dram_tensor("a_in", [R, CA], F32, kind="ExternalInput")
    s_in = nc.dram_tensor("s_in", [R, CS], F32, kind="ExternalInput")
    z_in = nc.dram_tensor("z_in", [R, S, CZ], BF, kind="ExternalInput")
    betaT_in = nc.dram_tensor("betaT_in", [H, R, S], BF, kind="ExternalInput")
    wz_in = nc.dram_tensor("wz_in", [CZ, NB * H], BF, kind="ExternalInput")
    sel_in = nc.dram_tensor("sel_in", [H, NB * H], BF, kind="ExternalInput")
    wsn_in = nc.dram_tensor("wsn_in", [CS, NB, 4 * CA], BF, kind="ExternalInput")
    wsr_in = nc.dram_tensor("wsr_in", [CS + 1, NB, 2 * CA], BF, kind="ExternalInput")
    wkv_in = nc.dram_tensor("wkv_in", [CA, NB, 2 * CA], BF, kind="ExternalInput")
    wqg_in = nc.dram_tensor("wqg_in", [CA, NB, 2 * CA], BF, kind="ExternalInput")
    bq_in = nc.dram_tensor("bq_in", [NB, CA], F32, kind="ExternalInput")
    wsw_in = nc.dram_tensor("wsw_in", [CA, NB, NHID], BF, kind="ExternalInput")
    wg2_in = nc.dram_tensor("wg2_in", [CA, NB, NHID], BF, kind="ExternalInput")
    wo_in = nc.dram_tensor("wo_in", [CA, NB, CA], BF, kind="ExternalInput")
    wout_in = nc.dram_tensor("wout_in", [NHID, NB, CA], BF, kind="ExternalInput")
    a_out = nc.dram_tensor("a_out", [R, CA], F32, kind="ExternalOutput")

    with tile.TileContext(nc) as tc, ExitStack() as ctx:
        const = ctx.enter_context(tc.tile_pool(name="const", bufs=1))
        ident = const.tile([128, 128], BF)
        make_identity(nc, ident)
        wz_sb = const.tile([CZ, NB * H], BF)
        nc.sync.dma_start(out=wz_sb, in_=wz_in[:])
        sel_sb = const.tile([H, NB * H], BF)
        nc.sync.dma_start(out=sel_sb, in_=sel_in[:])
        biases = const.tile([128, 1 + NB * H], F32)
        nc.vector.memset(biases[:, 0:1], EPS)
        for _bh in range(NB * H):
            nc.vector.memset(biases[:, 1 + _bh:2 + _bh], float(bias_pb[_bh]))

        pers = ctx.enter_context(tc.tile_pool(name="pers", bufs=1))
        a_sb = pers.tile([R, CA], F32)
        nc.sync.dma_start(out=a_sb, in_=a_in[:])

        # internal DRAM scratch
        dram = ctx.enter_context(tc.tile_pool(name="dram", bufs=1, space="DRAM"))
        zbeta_dr = dram.tile([NB * H, R, S], BF)

        # =========== s preprocessing (once) ===========
        with tc.tile_pool(name="sprep", bufs=1) as sp:
            s_sb = sp.tile([R, CS], F32)
            nc.sync.dma_start(out=s_sb, in_=s_in[:])
            stats = sp.tile([R, 1, 6], F32)
            mv = sp.tile([R, 2], F32)
            nc.vector.bn_stats(out=stats[:, 0, :], in_=s_sb[:])
            nc.vector.bn_aggr(out=mv, in_=stats)
            rstd = sp.tile([R, 1], F32)
            nc.scalar.activation(out=rstd, in_=mv[:, 1:2], func=ACTF.Sqrt,
                                 bias=biases[:, 0:1], scale=1.0)
            nc.vector.reciprocal(out=rstd, in_=rstd)
            s_n = sp.tile([R, CS], BF)
            nc.vector.tensor_scalar(s_n[:], s_sb[:], mv[:, 0:1], rstd[:, 0:1],
                                    OP.subtract, OP.mult)
            s_b16 = sp.tile([R, CS], BF)
            nc.vector.tensor_copy(s_b16[:], s_sb[:])

            # transposed copies (persistent for all blocks)
            s_nT = pers.tile([128, KT_S, 128], BF)
            sT = pers.tile([128, KT_S, 128], BF)
            ones_row = pers.tile([1, 128], BF)
            nc.vector.memset(ones_row, 1.0)
            with tc.tile_pool(name="tp_ps", bufs=2, space="PSUM") as tps:
                for kt in range(KT_S):
                    pt = tps.tile([128, 128], BF, tag="t")
                    nc.tensor.transpose(pt[:], s_n[:, bass.ts(kt, 128)], ident[:])
                    nc.scalar.copy(out=s_nT[:, kt, :], in_=pt[:])
                    pt2 = tps.tile([128, 128], BF, tag="t")
                    nc.tensor.transpose(pt2[:], s_b16[:, bass.ts(kt, 128)], ident[:])
                    nc.scalar.copy(out=sT[:, kt, :], in_=pt2[:])

            # per-block s-derived tensors: sg/sb for attn+tr, gates attn/tr
            sgsb = pers.tile([R, NB, 4 * CA], BF)   # wg_a|wb_a|wg_t|wb_t
            gts = pers.tile([R, NB, 2 * CA], BF)    # gate_attn|gate_tr
            with tc.tile_pool(name="sw", bufs=2) as swp, \
                 tc.tile_pool(name="sps", bufs=3, space="PSUM") as sps:
                for b in range(NB):
                    wsn_sb = swp.tile([128, KT_S, 4 * CA], BF, tag="wsn")
                    nc.sync.dma_start(
                        out=wsn_sb,
                        in_=wsn_in[:].rearrange("(kt p) b m -> p kt b m", p=128)[:, :, b, :])
                    for m in range(6):  # 3072 / 512
                        ps = sps.tile([R, 512], F32, tag="ps")
                        for kt in range(KT_S):
                            nc.tensor.matmul(ps[:], s_nT[:, kt, :],
                                             wsn_sb[:, kt, bass.ts(m, 512)],
                                             start=(kt == 0), stop=(kt == KT_S - 1))
                        # cols [m*512,(m+1)*512) of [wg_a(768)|wb_a|wg_t|wb_t]
                        for lo_, hi_ in [(m * 512, m * 512 + 256), (m * 512 + 256, (m + 1) * 512)]:
                            mat = lo_ // CA  # 0..3
                            f = ACTF.Sigmoid if mat in (0, 2) else ACTF.Copy
                            nc.scalar.activation(
                                out=sgsb[:, b, lo_:hi_], in_=ps[:, lo_ - m * 512:hi_ - m * 512],
                                func=f, bias=0.0 if f == ACTF.Copy else 0.0, scale=1.0)
                    wsr_sb = swp.tile([128, KT_S, 2 * CA], BF, tag="wsr")
                    nc.sync.dma_start(
                        out=wsr_sb,
                        in_=wsr_in[:CS].rearrange("(kt p) b m -> p kt b m", p=128)[:, :, b, :])
                    wsr_last = swp.tile([1, 2 * CA], BF, tag="wsrl")
                    nc.sync.dma_start(out=wsr_last, in_=wsr_in[CS:CS + 1, b, :])
                    for m in range(3):  # 1536 / 512
                        ps = sps.tile([R, 512], F32, tag="ps")
                        for kt in range(KT_S):
                            nc.tensor.matmul(ps[:], sT[:, kt, :],
                                             wsr_sb[:, kt, bass.ts(m, 512)],
                                             start=(kt == 0), stop=False)
                        nc.tensor.matmul(ps[:], ones_row[:],
                                         wsr_last[:, bass.ts(m, 512)],
                                         start=False, stop=True)
                        nc.scalar.activation(out=gts[:, b, bass.ts(m, 512)],
                                             in_=ps[:], func=ACTF.Sigmoid,
                                             bias=0.0, scale=1.0)

        # =========== z preprocessing (once) ===========
        with tc.tile_pool(name="zslab", bufs=4) as zsl, \
             tc.tile_pool(name="zsm", bufs=4) as zsm, \
             tc.tile_pool(name="ztp", bufs=3, space="PSUM") as ztp, \
             tc.tile_pool(name="zbp", bufs=2, space="PSUM") as zbp:
            z_r = z_in[:].rearrange("i (jt jp) c -> i jp jt c", jp=128)
            for i in range(R):
                zt = zsl.tile([128, JT, CZ], BF, tag="z")
                for _q in range(4):
                    nc.sync.dma_start(out=zt[:, 2 * _q:2 * _q + 2, :],
                                      in_=z_r[i][:, 2 * _q:2 * _q + 2, :])
                bsel = zsl.tile([H, S], BF, tag="bsel")
                nc.sync.dma_start(out=bsel, in_=betaT_in[:, i, :])
                st8 = zsm.tile([128, JT, 6], F32, tag="st")
                for jt in range(JT):
                    nc.vector.bn_stats(out=st8[:, jt, :], in_=zt[:, jt, :])
                # pooled even/odd moments, vectorized over all 8 j-tiles:
                # mean = (m_e+m_o)/2; var = (64v_e+64v_o)/128 + ((m_e-m_o)/2)^2
                mrow = zsm.tile([128, JT], F32, tag="mrow")
                nc.vector.tensor_tensor(mrow[:], st8[:, :, 1], st8[:, :, 4], OP.add)
                nc.vector.tensor_scalar(mrow[:], mrow[:], 0.5, None, OP.mult)
                dm = zsm.tile([128, JT], F32, tag="dm")
                nc.vector.tensor_tensor(dm[:], st8[:, :, 1], st8[:, :, 4],
                                        OP.subtract)
                nc.vector.tensor_tensor(dm[:], dm[:], dm[:], OP.mult)
                nc.vector.tensor_scalar(dm[:], dm[:], 0.25, None, OP.mult)
                var = zsm.tile([128, JT], F32, tag="var")
                nc.vector.tensor_tensor(var[:], st8[:, :, 2], st8[:, :, 5], OP.add)
                nc.vector.tensor_scalar(var[:], var[:], 1.0 / CZ, None, OP.mult)
                nc.vector.tensor_tensor(var[:], var[:], dm[:], OP.add)
                rst = zsm.tile([128, JT], F32, tag="rst")
                nc.scalar.activation(out=rst, in_=var[:], func=ACTF.Sqrt,
                                     bias=biases[:, 0:1], scale=1.0)
                nc.vector.reciprocal(out=rst, in_=rst)
                zh = zsm.tile([128, JT, CZ], BF, tag="zh")
                for jt in range(JT):
                    nc.vector.tensor_scalar(zh[:, jt, :], zt[:, jt, :],
                                            mrow[:, jt:jt + 1], rst[:, jt:jt + 1],
                                            OP.subtract, OP.mult)
                zhT = zsm.tile([128, JT, 128], BF, tag="zhT")
                for g in range(2):  # transpose 8 tiles, copy in 2 batches
                    pt = ztp.tile([128, 4, 128], BF, tag="t")
                    for q in range(4):
                        jt = g * 4 + q
                        nc.tensor.transpose(pt[:, q, :], zh[:, jt, :], ident[:])
                    nc.scalar.copy(out=zhT[:, g * 4:(g + 1) * 4, :], in_=pt[:])
                zb = zbp.tile([NB * H, S], F32, tag="zb")
                for jc in range(2):
                    nc.tensor.matmul(zb[:, bass.ts(jc, 512)], wz_sb[:],
                                     zhT[:].rearrange("p jt j -> p (jt j)")[:, bass.ts(jc, 512)],
                                     start=True, stop=False)
                    nc.tensor.matmul(zb[:, bass.ts(jc, 512)], sel_sb[:],
                                     bsel[:, bass.ts(jc, 512)],
                                     start=False, stop=True)
                zbs = zsm.tile([NB * H, S], BF, tag="zbs")
                nc.scalar.copy(out=zbs, in_=zb[:])
                nc.sync.dma_start(out=zbeta_dr[:, i, :], in_=zbs)

        # =========== block loop ===========
        wpool = ctx.enter_context(tc.tile_pool(name="wpool", bufs=2))
        wop = ctx.enter_context(tc.tile_pool(name="wop", bufs=1))
        blk = ctx.enter_context(tc.tile_pool(name="blk", bufs=1))
        kvg = ctx.enter_context(tc.tile_pool(name="kvg", bufs=1))
        att = ctx.enter_context(tc.tile_pool(name="att", bufs=3))
        dramc = ctx.enter_context(tc.tile_pool(name="dramc", bufs=2, space="DRAM"))

        for b in range(NB):
            # ---- ada_ln(a) shared stats ----
            with tc.tile_pool(name="lnp", bufs=1) as lnp, \
                 tc.tile_pool(name="ps_ln", bufs=2, space="PSUM") as pln:
                st3 = lnp.tile([R, 3, 6], F32)
                for g_ in range(3):
                    nc.vector.bn_stats(out=st3[:, g_, :],
                                       in_=a_sb[:, bass.ts(g_, 256)])
                mv = lnp.tile([R, 2], F32)
                nc.vector.bn_aggr(out=mv, in_=st3)
                rstd = lnp.tile([R, 1], F32)
                nc.scalar.activation(out=rstd, in_=mv[:, 1:2], func=ACTF.Sqrt,
                                     bias=biases[:, 0:1], scale=1.0)
                nc.vector.reciprocal(out=rstd, in_=rstd)
                xhat = lnp.tile([R, CA], BF)
                nc.vector.tensor_scalar(xhat[:], a_sb[:], mv[:, 0:1], rstd[:, 0:1],
                                        OP.subtract, OP.mult)
                ah = blk.tile([R, CA], BF, tag="ah")
                nc.vector.tensor_tensor(ah[:], xhat[:], sgsb[:, b, 0:CA], OP.mult)
                nc.vector.tensor_tensor(ah[:], ah[:], sgsb[:, b, CA:2 * CA], OP.add)
                th = blk.tile([R, CA], BF, tag="th")
                nc.vector.tensor_tensor(th[:], xhat[:], sgsb[:, b, 2 * CA:3 * CA], OP.mult)
                nc.vector.tensor_tensor(th[:], th[:], sgsb[:, b, 3 * CA:4 * CA], OP.add)
                ahT = blk.tile([128, KT_A, 128], BF, tag="ahT")
                for kt in range(KT_A):
                    pt = pln.tile([128, 128], BF, tag="t")
                    nc.tensor.transpose(pt[:], ah[:, bass.ts(kt, 128)], ident[:])
                    nc.scalar.copy(out=ahT[:, kt, :], in_=pt[:])
                thT = blk.tile([128, KT_A, 128], BF, tag="thT")
                for kt in range(KT_A):
                    pt = pln.tile([128, 128], BF, tag="t")
                    nc.tensor.transpose(pt[:], th[:, bass.ts(kt, 128)], ident[:])
                    nc.scalar.copy(out=thT[:, kt, :], in_=pt[:])

            # ---- kv local + gather ----
            kv_inb = dramc.tile([KV_TOT], BF, tag="kvin")
            kv_outb = dramc.tile([NCORE * KV_TOT], BF, tag="kvout",
                                 addr_space="Shared")
            with tc.tile_pool(name="ps_kv", bufs=3, space="PSUM") as pkv:
                wkv_sb = wpool.tile([128, KT_A, 2 * CA], BF, tag="w1536")
                nc.sync.dma_start(
                    out=wkv_sb,
                    in_=wkv_in[:].rearrange("(kt p) b m -> p kt b m", p=128)[:, :, b, :])
                kv_sb = kvg.tile([R, 2 * CA], BF, tag="kv")
                for m in range(3):  # 1536/512
                    ps = pkv.tile([R, 512], F32, tag="ps")
                    for kt in range(KT_A):
                        nc.tensor.matmul(ps[:], ahT[:, kt, :],
                                         wkv_sb[:, kt, bass.ts(m, 512)],
                                         start=(kt == 0), stop=(kt == KT_A - 1))
                    nc.scalar.copy(out=kv_sb[:, bass.ts(m, 512)], in_=ps[:])
                # kT head-tiles
                kT_loc = kvg.tile([D, H, 128], BF, tag="kT")
                for h in range(H):
                    pt = pkv.tile([128, 128], BF, tag="t")
                    nc.tensor.transpose(pt[:D, :], kv_sb[:, h * D:(h + 1) * D], ident[:])
                    nc.vector.tensor_copy(kT_loc[:, h, :], pt[:D, :])
                nc.sync.dma_start(out=kv_inb[0:KV_K].rearrange("(d x) -> d x", d=D),
                                  in_=kT_loc[:].rearrange("d h j -> d (h j)"))
                nc.sync.dma_start(out=kv_inb[KV_K:].rearrange("(j c) -> j c", j=R),
                                  in_=kv_sb[:, CA:])
            nc.gpsimd.collective_compute(
                "AllGather", OP.bypass,
                replica_groups=[list(range(NCORE))],
                ins=[kv_inb[:].opt()], outs=[kv_outb[:].opt()])

            # ---- qT, gT (transposed head layout) ----
            with tc.tile_pool(name="ps_qg", bufs=3, space="PSUM") as pqg:
                wqg_sb = wpool.tile([128, KT_A, 2 * CA], BF, tag="w1536")
                nc.sync.dma_start(
                    out=wqg_sb,
                    in_=wqg_in[:].rearrange("(kt p) b m -> p kt b m", p=128)[:, :, b, :])
                bq_sb = blk.tile([D, H], F32, tag="bq")
                nc.sync.dma_start(
                    out=bq_sb, in_=bq_in[b].rearrange("(h d) -> d h", d=D))
                qT = blk.tile([D, H, 128], BF, tag="qT")
                gT = blk.tile([D, H, 128], BF, tag="gT")
                for h in range(H):
                    ps = pqg.tile([D, 128], F32, tag="ps")
                    for kt in range(KT_A):
                        nc.tensor.matmul(ps[:], wqg_sb[:, kt, h * D:(h + 1) * D],
                                         ahT[:, kt, :],
                                         start=(kt == 0), stop=(kt == KT_A - 1))
                    nc.scalar.activation(out=qT[:, h, :], in_=ps[:],
                                         func=ACTF.Identity,
                                         bias=bq_sb[:, h:h + 1], scale=1.0)
                    ps2 = pqg.tile([D, 128], F32, tag="ps")
                    for kt in range(KT_A):
                        nc.tensor.matmul(ps2[:], wqg_sb[:, kt, CA + h * D:CA + (h + 1) * D],
                                         ahT[:, kt, :],
                                         start=(kt == 0), stop=(kt == KT_A - 1))
                    nc.scalar.activation(out=gT[:, h, :], in_=ps2[:],
                                         func=ACTF.Sigmoid, bias=0.0, scale=1.0)

            # ---- transition: hiddenT = silu(th@wsw)^T * (th@wg2)^T ----
            hidT = blk.tile([128, KT_H, 128], BF, tag="hidT")
            with tc.tile_pool(name="ps_h", bufs=3, space="PSUM") as psh:
                wsw_sb = wpool.tile([128, KT_A, NHID], BF, tag="w1536")
                nc.sync.dma_start(
                    out=wsw_sb,
                    in_=wsw_in[:].rearrange("(kt p) b m -> p kt b m", p=128)[:, :, b, :])
                swT = blk.tile([128, KT_H, 128], BF, tag="swT")
                for mt in range(KT_H):
                    ps = psh.tile([128, 128], F32, tag="ps")
                    for kt in range(KT_A):
                        nc.tensor.matmul(ps[:], wsw_sb[:, kt, bass.ts(mt, 128)],
                                         thT[:, kt, :],
                                         start=(kt == 0), stop=(kt == KT_A - 1))
                    nc.scalar.activation(out=swT[:, mt, :], in_=ps[:],
                                         func=ACTF.Silu, bias=0.0, scale=1.0)
                wg2_sb = wpool.tile([128, KT_A, NHID], BF, tag="w1536")
                nc.sync.dma_start(
                    out=wg2_sb,
                    in_=wg2_in[:].rearrange("(kt p) b m -> p kt b m", p=128)[:, :, b, :])
                for mt in range(KT_H):
                    ps = psh.tile([128, 128], F32, tag="ps")
                    for kt in range(KT_A):
                        nc.tensor.matmul(ps[:], wg2_sb[:, kt, bass.ts(mt, 128)],
                                         thT[:, kt, :],
                                         start=(kt == 0), stop=(kt == KT_A - 1))
                    g2 = att.tile([128, 128], BF, tag="g2")
                    nc.scalar.copy(out=g2, in_=ps[:])
                    nc.vector.tensor_tensor(hidT[:, mt, :], swT[:, mt, :], g2[:],
                                            OP.mult)

            # ---- attention ----
            kv_outr = kv_outb[:].rearrange("(r x) -> r x", r=NCORE)
            kT_src = kv_outr[:, 0:KV_K].rearrange(
                "r (d h j) -> d h r j", d=D, h=H)
            v_full = kvg.tile([128, NCORE, CA], BF, tag="vf")
            v_src = kv_outb[:].rearrange("(r x) -> r x", r=NCORE)[:, KV_K:] \
                .rearrange("r (j c) -> j r c", j=R)
            for _q in range(4):
                nc.sync.dma_start(out=v_full[:, 2 * _q:2 * _q + 2, :],
                                  in_=v_src[:, 2 * _q:2 * _q + 2, :])
            go_T = blk.tile([D, H, 128], BF, tag="goT")
            sums = blk.tile([R, H], F32, tag="sums")
            with tc.tile_pool(name="ps_s", bufs=2, space="PSUM") as pss, \
                 tc.tile_pool(name="ps_t", bufs=2, space="PSUM") as pst, \
                 tc.tile_pool(name="ps_o", bufs=2, space="PSUM") as pso:
                for h in range(H):
                    kT_h = att.tile([D, NCORE, 128], BF, tag="kTh")
                    nc.sync.dma_start(out=kT_h[:, 0:4, :], in_=kT_src[:, h, 0:4, :])
                    nc.sync.dma_start(out=kT_h[:, 4:8, :], in_=kT_src[:, h, 4:8, :])
                    ps_s = pss.tile([R, S], F32, tag="s")
                    for jc in range(2):
                        nc.tensor.matmul(ps_s[:, bass.ts(jc, 512)], qT[:, h, :],
                                         kT_h[:, jc * 4:(jc + 1) * 4, :],
                                         start=True, stop=True)
                    zb_t = att.tile([R, S], BF, tag="zbt")
                    nc.sync.dma_start(out=zb_t[:, 0:512],
                                      in_=zbeta_dr[b * H + h, :, 0:512])
                    nc.sync.dma_start(out=zb_t[:, 512:1024],
                                      in_=zbeta_dr[b * H + h, :, 512:1024])
                    nc.vector.tensor_tensor(ps_s[:], ps_s[:], zb_t[:], OP.add)
                    attn = att.tile([R, S], BF, tag="attn")
                    bh_ = 1 + b * H + h
                    nc.scalar.activation(out=attn, in_=ps_s[:], func=ACTF.Exp,
                                         bias=biases[:, bh_:bh_ + 1], scale=1.0,
                                         accum_out=sums[:, h:h + 1])
                    rec = att.tile([R, 1], F32, tag="rec")
                    nc.vector.reciprocal(out=rec, in_=sums[:, h:h + 1])
                    nc.vector.tensor_scalar(attn[:], attn[:], rec[:, 0:1], None,
                                            OP.mult)
                    attnT = att.tile([128, JT, 128], BF, tag="attnT")
                    for g in range(2):
                        pt = pst.tile([128, 4, 128], BF, tag="t")
                        for q in range(4):
                            jt = g * 4 + q
                            nc.tensor.transpose(pt[:, q, :],
                                                attn[:, bass.ts(jt, 128)], ident[:])
                        nc.vector.tensor_copy(attnT[:, g * 4:(g + 1) * 4, :], pt[:])
                    ps_o = pso.tile([128, 128], F32, tag="o")
                    for jt in range(JT):
                        nc.tensor.matmul(ps_o[:D, :], v_full[:, jt, h * D:(h + 1) * D],
                                         attnT[:, jt, :],
                                         start=(jt == 0), stop=(jt == JT - 1))
                    nc.vector.tensor_tensor(go_T[:, h, :], ps_o[:D, :],
                                            gT[:, h, :], OP.mult)

            # ---- att_out = (g*o) @ wo ; b_attn = gate_attn * att_out ----
            b_attn = blk.tile([R, CA], F32, tag="batt")
            with tc.tile_pool(name="ps_wo", bufs=2, space="PSUM") as pwo:
                wo_sb = wop.tile([D, H, CA], BF, tag="wo")
                nc.sync.dma_start(
                    out=wo_sb,
                    in_=wo_in[:].rearrange("(h d) b m -> d h b m", d=D)[:, :, b, :])
                for m in range(2):
                    n0, n1 = (0, 512) if m == 0 else (512, 768)
                    ps = pwo.tile([R, 512], F32, tag="ps")
                    for h in range(H):
                        nc.tensor.matmul(ps[:, 0:n1 - n0], go_T[:, h, :],
                                         wo_sb[:, h, n0:n1],
                                         start=(h == 0), stop=(h == H - 1))
                    nc.vector.tensor_tensor(b_attn[:, n0:n1], ps[:, 0:n1 - n0],
                                            gts[:, b, n0:n1], OP.mult)

            # ---- tr = gate_tr * (hidden @ w_out); a = b_attn + tr ----
            with tc.tile_pool(name="ps_tr", bufs=2, space="PSUM") as ptr:
                wout_sb = wop.tile([128, KT_H, CA], BF, tag="wout")
                nc.sync.dma_start(
                    out=wout_sb,
                    in_=wout_in[:].rearrange("(kt p) b m -> p kt b m", p=128)[:, :, b, :])
                for m in range(2):
                    n0, n1 = (0, 512) if m == 0 else (512, 768)
                    ps = ptr.tile([R, 512], F32, tag="ps")
                    for kt in range(KT_H):
                        nc.tensor.matmul(ps[:, 0:n1 - n0], hidT[:, kt, :],
                                         wout_sb[:, kt, n0:n1],
                                         start=(kt == 0), stop=(kt == KT_H - 1))
                    tr = att.tile([R, 512], F32, tag="tr")
                    nc.vector.tensor_tensor(tr[:, 0:n1 - n0], ps[:, 0:n1 - n0],
                                            gts[:, b, CA + n0:CA + n1], OP.mult)
                    nc.vector.tensor_tensor(a_sb[:, n0:n1], b_attn[:, n0:n1],
                                            tr[:, 0:n1 - n0], OP.add)

        nc.sync.dma_start(out=a_out[:], in_=a_sb[:])

    nc.finalize()
    return nc


def _prep_inputs(a, s, z, beta, ln_s_w_attn, wg_attn, wb_attn, wq, bq, wk, wv,
                 ln_z_w, ln_z_b, wpb, wgate, wo, wsg_attn, bsg_attn,
                 ln_s_w_tr, wg_tr, wb_tr, w_swish, w_gate2, wsg_tr, bsg_tr, w_out):
    bf = ml_dtypes.bfloat16
    f32 = np.float32
    scale = 1.0 / np.sqrt(np.float32(D))

    # folded weights (shared across cores)
    wz = np.concatenate([ln_z_w[i][:, None] * wpb[i] for i in range(NB)],
                        axis=1).astype(bf)                       # [CZ, NB*H]
    bias_pb = np.concatenate([ln_z_b[i] @ wpb[i] for i in range(NB)])  # [NB*H]
    sel = np.tile(np.eye(H, dtype=np.float32), (1, NB)).astype(bf)  # [H, NB*H]
    wsn = np.stack([np.concatenate(
        [ln_s_w_attn[i][:, None] * wg_attn[i], ln_s_w_attn[i][:, None] * wb_attn[i],
         ln_s_w_tr[i][:, None] * wg_tr[i], ln_s_w_tr[i][:, None] * wb_tr[i]],
        axis=1) for i in range(NB)], axis=1).astype(bf)          # [CS, NB, 4CA]
    wsr = np.stack([np.concatenate(
        [np.concatenate([wsg_attn[i], bsg_attn[i][None, :]], 0),
         np.concatenate([wsg_tr[i], bsg_tr[i][None, :]], 0)], axis=1)
        for i in range(NB)], axis=1).astype(bf)                  # [CS+1, NB, 2CA]
    wkv = np.stack([np.concatenate([wk[i], wv[i]], 1) for i in range(NB)],
                   axis=1).astype(bf)                            # [CA, NB, 2CA]
    wqg = np.stack([np.concatenate([wq[i] * scale, wgate[i]], 1)
                    for i in range(NB)], axis=1).astype(bf)
    bqe = (bq * scale).astype(f32)                               # [NB, CA]
    wsw = np.stack([w_swish[i] for i in range(NB)], axis=1).astype(bf)
    wg2 = np.stack([w_gate2[i] for i in range(NB)], axis=1).astype(bf)
    wob = np.stack([wo[i] for i in range(NB)], axis=1).astype(bf)
    wout = np.stack([w_out[i] for i in range(NB)], axis=1).astype(bf)

    shared = dict(wz_in=np.ascontiguousarray(wz),
                  sel_in=np.ascontiguousarray(sel),
                  wsn_in=np.ascontiguousarray(wsn),
                  wsr_in=np.ascontiguousarray(wsr),
                  wkv_in=np.ascontiguousarray(wkv),
                  wqg_in=np.ascontiguousarray(wqg),
                  bq_in=np.ascontiguousarray(bqe),
                  wsw_in=np.ascontiguousarray(wsw),
                  wg2_in=np.ascontiguousarray(wg2),
                  wo_in=np.ascontiguousarray(wob),
                  wout_in=np.ascontiguousarray(wout))

    a2 = a.reshape(S, CA).astype(f32)
    s2 = s.reshape(S, CS).astype(f32)
    z2 = z.reshape(S, S, CZ).astype(bf)
    betaT = np.ascontiguousarray(
        beta.reshape(S, S, H).transpose(2, 0, 1)).astype(bf)     # [H, S, S]

    in_maps = []
    for c in range(NCORE):
        rows = slice(c * R, (c + 1) * R)
        m = dict(shared)
        m["a_in"] = np.ascontiguousarray(a2[rows])
        m["s_in"] = np.ascontiguousarray(s2[rows])
        m["z_in"] = np.ascontiguousarray(z2[rows])
        m["betaT_in"] = np.ascontiguousarray(betaT[:, rows, :])
        in_maps.append(m)
    return in_maps, [float(x) for x in bias_pb]


_CACHE = {}


def kernel(**inputs):
    inputs = {k: np.asarray(v) for k, v in inputs.items()}
    in_maps, bias_pb = _prep_inputs(**inputs)
    key = tuple(bias_pb)
    if key not in _CACHE:
        _CACHE.clear()
        _CACHE[key] = build_program(bias_pb)
    nc = _CACHE[key]
    res = run_bass_kernel_spmd(nc, in_maps, core_ids=list(range(NCORE)),
                               trace=False)
    out = np.concatenate([res.results[c]["a_out"] for c in range(NCORE)], axis=0)
    return out.reshape(1, S, CA).astype(np.float32)


if __name__ == "__main__":
    import reference
    ins = {k: np.asarray(v) for k, v in reference.setup_inputs().items()}
    exp = np.asarray(reference.reference(**ins))
    act = kernel(**ins)
    err = np.abs(act - exp).max() / (np.abs(exp).max() + 1e-9)
    print("rel err:", err)



# revision 37
# speedup vs baseline: 6.2862x; 4.6423x over previous
"""DiffusionTransformer (AF3-style) Trainium2 kernel, 8-core SPMD.

Sharding: sequence-parallel over rows (queries). Each core owns R=128 rows of
a / z / beta. Per block, k/v are computed on local rows and AllGathered
(bf16, ~384KB per rank). zbeta (pair bias LN(z)@wpb + beta, all 4 blocks) is
precomputed once into per-core DRAM in [bh, i, j] layout.
"""
import numpy as np
import ml_dtypes
from contextlib import ExitStack

import concourse.bass as bass
import concourse.mybir as mybir
import concourse.tile as tile
from concourse import bacc
from concourse.bass_utils import run_bass_kernel_spmd
from concourse.masks import make_identity

NB, H, S, CA, CS, CZ = 4, 16, 1024, 768, 384, 128
D = CA // H            # 48
NCORE = 8
R = S // NCORE         # 128 rows per core
NHID = 2 * CA          # 1536
EPS = 1e-5
BF = mybir.dt.bfloat16
F32 = mybir.dt.float32
AX = mybir.AxisListType
OP = mybir.AluOpType
ACTF = mybir.ActivationFunctionType
JT = S // 128          # 8 j-tiles
KT_A = CA // 128       # 6 k-tiles over c_a
KT_S = CS // 128       # 3 k-tiles over c_s
KT_H = NHID // 128     # 12 k-tiles over n_hidden
KV_K = D * H * R       # 98304 flat elems of kT part
KV_TOT = KV_K + R * CA  # + v part


def _headsplit_ranges():
    """Split [0,768) hd-range at both 128-tile and 48-head boundaries.

    Returns list of (kt, p0, p1, h, d0) with kt*128+p0 == h*48+d0.
    """
    out = []
    bounds = sorted(set([x * 128 for x in range(KT_A + 1)] +
                        [h * D for h in range(H + 1)]))
    for lo, hi in zip(bounds[:-1], bounds[1:]):
        kt, p0 = lo // 128, lo % 128
        h, d0 = lo // D, lo % D
        out.append((kt, p0, hi - lo, h, d0))
    return out


HS = _headsplit_ranges()


def build_program(bias_pb):
    nc = bacc.Bacc("TRN2", target_bir_lowering=False, debug=False,
                   num_devices=NCORE)

    # ---------------- I/O ----------------
    a_in = nc.dram_tensor("a_in", [R, CA], F32, kind="ExternalInput")
    s_in = nc.dram_tensor("s_in", [R, CS], F32, kind="ExternalInput")
    z_in = nc.dram_tensor("z_in", [R, 128, JT, CZ], BF, kind="ExternalInput")
    betaT_in = nc.dram_tensor("betaT_in", [H, R, S], BF, kind="ExternalInput")
    wz_in = nc.dram_tensor("wz_in", [CZ, NB * H], BF, kind="ExternalInput")
    sel_in = nc.dram_tensor("sel_in", [H, NB * H], BF, kind="ExternalInput")
    wsn_in = nc.dram_tensor("wsn_in", [CS, NB, 4 * CA], BF, kind="ExternalInput")
    wsr_in = nc.dram_tensor("wsr_in", [CS + 1, NB, 2 * CA], BF, kind="ExternalInput")
    wkv_in = nc.dram_tensor("wkv_in", [CA, NB, 2 * CA], BF, kind="ExternalInput")
    wqg_in = nc.dram_tensor("wqg_in", [CA, NB, 2 * CA], BF, kind="ExternalInput")
    bq_in = nc.dram_tensor("bq_in", [NB, CA], F32, kind="ExternalInput")
    wsw_in = nc.dram_tensor("wsw_in", [CA, NB, NHID], BF, kind="ExternalInput")
    wg2_in = nc.dram_tensor("wg2_in", [CA, NB, NHID], BF, kind="ExternalInput")
    wo_in = nc.dram_tensor("wo_in", [CA, NB, CA], BF, kind="ExternalInput")
    wout_in = nc.dram_tensor("wout_in", [NHID, NB, CA], BF, kind="ExternalInput")
    a_out = nc.dram_tensor("a_out", [R, CA], F32, kind="ExternalOutput")

    with tile.TileContext(nc) as tc, ExitStack() as ctx:
        const = ctx.enter_context(tc.tile_pool(name="const", bufs=1))
        ident = const.tile([128, 128], BF)
        make_identity(nc, ident)
        wz_sb = const.tile([CZ, NB * H], BF)
        nc.sync.dma_start(out=wz_sb, in_=wz_in[:])
        sel_sb = const.tile([H, NB * H], BF)
        nc.sync.dma_start(out=sel_sb, in_=sel_in[:])
        biases = const.tile([128, 1 + NB * H], F32)
        nc.vector.memset(biases[:, 0:1], EPS)
        for _bh in range(NB * H):
            nc.vector.memset(biases[:, 1 + _bh:2 + _bh], float(bias_pb[_bh]))

        pers = ctx.enter_context(tc.tile_pool(name="pers", bufs=1))
        a_sb = pers.tile([R, CA], F32)
        nc.sync.dma_start(out=a_sb, in_=a_in[:])

        # internal DRAM scratch
        dram = ctx.enter_context(tc.tile_pool(name="dram", bufs=1, space="DRAM"))
        zbeta_dr = dram.tile([NB * H, R, S], BF)

        # =========== s preprocessing (once) ===========
        with tc.tile_pool(name="sprep", bufs=1) as sp:
            s_sb = sp.tile([R, CS], F32)
            nc.sync.dma_start(out=s_sb, in_=s_in[:])
            stats = sp.tile([R, 1, 6], F32)
            mv = sp.tile([R, 2], F32)
            nc.vector.bn_stats(out=stats[:, 0, :], in_=s_sb[:])
            nc.vector.bn_aggr(out=mv, in_=stats)
            rstd = sp.tile([R, 1], F32)
            nc.scalar.activation(out=rstd, in_=mv[:, 1:2], func=ACTF.Sqrt,
                                 bias=biases[:, 0:1], scale=1.0)
            nc.vector.reciprocal(out=rstd, in_=rstd)
            s_n = sp.tile([R, CS], BF)
            nc.vector.tensor_scalar(s_n[:], s_sb[:], mv[:, 0:1], rstd[:, 0:1],
                                    OP.subtract, OP.mult)
            s_b16 = sp.tile([R, CS], BF)
            nc.vector.tensor_copy(s_b16[:], s_sb[:])

            # transposed copies (persistent for all blocks)
            s_nT = pers.tile([128, KT_S, 128], BF)
            sT = pers.tile([128, KT_S, 128], BF)
            ones_row = pers.tile([1, 128], BF)
            nc.vector.memset(ones_row, 1.0)
            with tc.tile_pool(name="tp_ps", bufs=2, space="PSUM") as tps:
                for kt in range(KT_S):
                    pt = tps.tile([128, 128], BF, tag="t")
                    nc.tensor.transpose(pt[:], s_n[:, bass.ts(kt, 128)], ident[:])
                    nc.scalar.copy(out=s_nT[:, kt, :], in_=pt[:])
                    pt2 = tps.tile([128, 128], BF, tag="t")
                    nc.tensor.transpose(pt2[:], s_b16[:, bass.ts(kt, 128)], ident[:])
                    nc.scalar.copy(out=sT[:, kt, :], in_=pt2[:])

            # per-block s-derived tensors: sg/sb for attn+tr, gates attn/tr
            sgsb = pers.tile([R, NB, 4 * CA], BF)   # wg_a|wb_a|wg_t|wb_t
            gts = pers.tile([R, NB, 2 * CA], BF)    # gate_attn|gate_tr
            with tc.tile_pool(name="sw", bufs=2) as swp, \
                 tc.tile_pool(name="sps", bufs=3, space="PSUM") as sps:
                for b in range(NB):
                    wsn_sb = swp.tile([128, KT_S, 4 * CA], BF, tag="wsn")
                    nc.sync.dma_start(
                        out=wsn_sb,
                        in_=wsn_in[:].rearrange("(kt p) b m -> p kt b m", p=128)[:, :, b, :])
                    for m in range(6):  # 3072 / 512
                        ps = sps.tile([R, 512], F32, tag="ps")
                        for kt in range(KT_S):
                            nc.tensor.matmul(ps[:], s_nT[:, kt, :],
                                             wsn_sb[:, kt, bass.ts(m, 512)],
                                             start=(kt == 0), stop=(kt == KT_S - 1))
                        # cols [m*512,(m+1)*512) of [wg_a(768)|wb_a|wg_t|wb_t]
                        for lo_, hi_ in [(m * 512, m * 512 + 256), (m * 512 + 256, (m + 1) * 512)]:
                            mat = lo_ // CA  # 0..3
                            f = ACTF.Sigmoid if mat in (0, 2) else ACTF.Copy
                            nc.scalar.activation(
                                out=sgsb[:, b, lo_:hi_], in_=ps[:, lo_ - m * 512:hi_ - m * 512],
                                func=f, bias=0.0 if f == ACTF.Copy else 0.0, scale=1.0)
                    wsr_sb = swp.tile([128, KT_S, 2 * CA], BF, tag="wsr")
                    nc.sync.dma_start(
                        out=wsr_sb,
                        in_=wsr_in[:CS].rearrange("(kt p) b m -> p kt b m", p=128)[:, :, b, :])
                    wsr_last = swp.tile([1, 2 * CA], BF, tag="wsrl")
                    nc.sync.dma_start(out=wsr_last, in_=wsr_in[CS:CS + 1, b, :])
                    for m in range(3):  # 1536 / 512
                        ps = sps.tile([R, 512], F32, tag="ps")
                        for kt in range(KT_S):
                            nc.tensor.matmul(ps[:], sT[:, kt, :],
                                             wsr_sb[:, kt, bass.ts(m, 512)],
                                             start=(kt == 0), stop=False)
                        nc.tensor.matmul(ps[:], ones_row[:],
                                         wsr_last[:, bass.ts(m, 512)],
                                         start=False, stop=True)
                        nc.scalar.activation(out=gts[:, b, bass.ts(m, 512)],
                                             in_=ps[:], func=ACTF.Sigmoid,
                                             bias=0.0, scale=1.0)

        # =========== z preprocessing (once) ===========
        with tc.tile_pool(name="zslab", bufs=4) as zsl, \
             tc.tile_pool(name="zsm", bufs=4) as zsm, \
             tc.tile_pool(name="ztp", bufs=3, space="PSUM") as ztp, \
             tc.tile_pool(name="zbp", bufs=2, space="PSUM") as zbp:
            for i in range(R):
                zt = zsl.tile([128, JT, CZ], BF, tag="z")
                nc.sync.dma_start(out=zt, in_=z_in[i])
                bsel = zsl.tile([H, S], BF, tag="bsel")
                nc.sync.dma_start(out=bsel, in_=betaT_in[:, i, :])
                st8 = zsm.tile([128, JT, 6], F32, tag="st")
                for jt in range(JT):
                    nc.vector.bn_stats(out=st8[:, jt, :], in_=zt[:, jt, :])
                # pooled even/odd moments, vectorized over all 8 j-tiles:
                # mean = (m_e+m_o)/2; var = (64v_e+64v_o)/128 + ((m_e-m_o)/2)^2
                mrow = zsm.tile([128, JT], F32, tag="mrow")
                nc.vector.tensor_tensor(mrow[:], st8[:, :, 1], st8[:, :, 4], OP.add)
                nc.vector.tensor_scalar(mrow[:], mrow[:], 0.5, None, OP.mult)
                dm = zsm.tile([128, JT], F32, tag="dm")
                nc.vector.tensor_tensor(dm[:], st8[:, :, 1], st8[:, :, 4],
                                        OP.subtract)
                nc.vector.tensor_tensor(dm[:], dm[:], dm[:], OP.mult)
                nc.vector.tensor_scalar(dm[:], dm[:], 0.25, None, OP.mult)
                var = zsm.tile([128, JT], F32, tag="var")
                nc.vector.tensor_tensor(var[:], st8[:, :, 2], st8[:, :, 5], OP.add)
                nc.vector.tensor_scalar(var[:], var[:], 1.0 / CZ, None, OP.mult)
                nc.vector.tensor_tensor(var[:], var[:], dm[:], OP.add)
                rst = zsm.tile([128, JT], F32, tag="rst")
                nc.scalar.activation(out=rst, in_=var[:], func=ACTF.Sqrt,
                                     bias=biases[:, 0:1], scale=1.0)
                nc.vector.reciprocal(out=rst, in_=rst)
                zh = zsm.tile([128, JT, CZ], BF, tag="zh")
                for jt in range(JT):
                    nc.vector.tensor_scalar(zh[:, jt, :], zt[:, jt, :],
                                            mrow[:, jt:jt + 1], rst[:, jt:jt + 1],
                                            OP.subtract, OP.mult)
                zhT = zsm.tile([128, JT, 128], BF, tag="zhT")
                for g in range(2):  # transpose 8 tiles, copy in 2 batches
                    pt = ztp.tile([128, 4, 128], BF, tag="t")
                    for q in range(4):
                        jt = g * 4 + q
                        nc.tensor.transpose(pt[:, q, :], zh[:, jt, :], ident[:])
                    nc.scalar.copy(out=zhT[:, g * 4:(g + 1) * 4, :], in_=pt[:])
                zb = zbp.tile([NB * H, S], F32, tag="zb")
                for jc in range(2):
                    nc.tensor.matmul(zb[:, bass.ts(jc, 512)], wz_sb[:],
                                     zhT[:].rearrange("p jt j -> p (jt j)")[:, bass.ts(jc, 512)],
                                     start=True, stop=False)
                    nc.tensor.matmul(zb[:, bass.ts(jc, 512)], sel_sb[:],
                                     bsel[:, bass.ts(jc, 512)],
                                     start=False, stop=True)
                zbs = zsm.tile([NB * H, S], BF, tag="zbs")
                nc.scalar.copy(out=zbs, in_=zb[:])
                nc.sync.dma_start(out=zbeta_dr[:, i, :], in_=zbs)

        # =========== block loop ===========
        wpool = ctx.enter_context(tc.tile_pool(name="wpool", bufs=2))
        wop = ctx.enter_context(tc.tile_pool(name="wop", bufs=1))
        blk = ctx.enter_context(tc.tile_pool(name="blk", bufs=1))
        kvg = ctx.enter_context(tc.tile_pool(name="kvg", bufs=1))
        att = ctx.enter_context(tc.tile_pool(name="att", bufs=3))
        dramc = ctx.enter_context(tc.tile_pool(name="dramc", bufs=2, space="DRAM"))

        for b in range(NB):
            # ---- ada_ln(a) shared stats ----
            with tc.tile_pool(name="lnp", bufs=1) as lnp, \
                 tc.tile_pool(name="ps_ln", bufs=2, space="PSUM") as pln:
                st3 = lnp.tile([R, 3, 6], F32)
                for g_ in range(3):
                    nc.vector.bn_stats(out=st3[:, g_, :],
                                       in_=a_sb[:, bass.ts(g_, 256)])
                mv = lnp.tile([R, 2], F32)
                nc.vector.bn_aggr(out=mv, in_=st3)
                rstd = lnp.tile([R, 1], F32)
                nc.scalar.activation(out=rstd, in_=mv[:, 1:2], func=ACTF.Sqrt,
                                     bias=biases[:, 0:1], scale=1.0)
                nc.vector.reciprocal(out=rstd, in_=rstd)
                xhat = lnp.tile([R, CA], BF)
                nc.vector.tensor_scalar(xhat[:], a_sb[:], mv[:, 0:1], rstd[:, 0:1],
                                        OP.subtract, OP.mult)
                ah = blk.tile([R, CA], BF, tag="ah")
                nc.vector.tensor_tensor(ah[:], xhat[:], sgsb[:, b, 0:CA], OP.mult)
                nc.vector.tensor_tensor(ah[:], ah[:], sgsb[:, b, CA:2 * CA], OP.add)
                th = blk.tile([R, CA], BF, tag="th")
                nc.vector.tensor_tensor(th[:], xhat[:], sgsb[:, b, 2 * CA:3 * CA], OP.mult)
                nc.vector.tensor_tensor(th[:], th[:], sgsb[:, b, 3 * CA:4 * CA], OP.add)
                ahT = blk.tile([128, KT_A, 128], BF, tag="ahT")
                for kt in range(KT_A):
                    pt = pln.tile([128, 128], BF, tag="t")
                    nc.tensor.transpose(pt[:], ah[:, bass.ts(kt, 128)], ident[:])
                    nc.scalar.copy(out=ahT[:, kt, :], in_=pt[:])
                thT = blk.tile([128, KT_A, 128], BF, tag="thT")
                for kt in range(KT_A):
                    pt = pln.tile([128, 128], BF, tag="t")
                    nc.tensor.transpose(pt[:], th[:, bass.ts(kt, 128)], ident[:])
                    nc.scalar.copy(out=thT[:, kt, :], in_=pt[:])

            # ---- kv local + gather ----
            kv_inb = dramc.tile([KV_TOT], BF, tag="kvin")
            kv_outb = dramc.tile([NCORE * KV_TOT], BF, tag="kvout",
                                 addr_space="Shared")
            with tc.tile_pool(name="ps_kv", bufs=3, space="PSUM") as pkv:
                wkv_sb = wpool.tile([128, KT_A, 2 * CA], BF, tag="w1536")
                nc.sync.dma_start(
                    out=wkv_sb,
                    in_=wkv_in[:].rearrange("(kt p) b m -> p kt b m", p=128)[:, :, b, :])
                kv_sb = kvg.tile([R, 2 * CA], BF, tag="kv")
                for m in range(3):  # 1536/512
                    ps = pkv.tile([R, 512], F32, tag="ps")
                    for kt in range(KT_A):
                        nc.tensor.matmul(ps[:], ahT[:, kt, :],
                                         wkv_sb[:, kt, bass.ts(m, 512)],
                                         start=(kt == 0), stop=(kt == KT_A - 1))
                    nc.scalar.copy(out=kv_sb[:, bass.ts(m, 512)], in_=ps[:])
                # kT head-tiles
                kT_loc = kvg.tile([D, H, 128], BF, tag="kT")
                for h in range(H):
                    pt = pkv.tile([128, 128], BF, tag="t")
                    nc.tensor.transpose(pt[:D, :], kv_sb[:, h * D:(h + 1) * D], ident[:])
                    nc.vector.tensor_copy(kT_loc[:, h, :], pt[:D, :])
                nc.sync.dma_start(out=kv_inb[0:KV_K].rearrange("(d x) -> d x", d=D),
                                  in_=kT_loc[:].rearrange("d h j -> d (h j)"))
                nc.sync.dma_start(out=kv_inb[KV_K:].rearrange("(j c) -> j c", j=R),
                                  in_=kv_sb[:, CA:])
            nc.gpsimd.collective_compute(
                "AllGather", OP.bypass,
                replica_groups=[list(range(NCORE))],
                ins=[kv_inb[:].opt()], outs=[kv_outb[:].opt()])

            # ---- qT, gT (transposed head layout) ----
            with tc.tile_pool(name="ps_qg", bufs=3, space="PSUM") as pqg:
                wqg_sb = wpool.tile([128, KT_A, 2 * CA], BF, tag="w1536")
                nc.sync.dma_start(
                    out=wqg_sb,
                    in_=wqg_in[:].rearrange("(kt p) b m -> p kt b m", p=128)[:, :, b, :])
                bq_sb = blk.tile([D, H], F32, tag="bq")
                nc.sync.dma_start(
                    out=bq_sb, in_=bq_in[b].rearrange("(h d) -> d h", d=D))
                qT = blk.tile([D, H, 128], BF, tag="qT")
                gT = blk.tile([D, H, 128], BF, tag="gT")
                for h in range(H):
                    ps = pqg.tile([D, 128], F32, tag="ps")
                    for kt in range(KT_A):
                        nc.tensor.matmul(ps[:], wqg_sb[:, kt, h * D:(h + 1) * D],
                                         ahT[:, kt, :],
                                         start=(kt == 0), stop=(kt == KT_A - 1))
                    nc.scalar.activation(out=qT[:, h, :], in_=ps[:],
                                         func=ACTF.Identity,
                                         bias=bq_sb[:, h:h + 1], scale=1.0)
                    ps2 = pqg.tile([D, 128], F32, tag="ps")
                    for kt in range(KT_A):
                        nc.tensor.matmul(ps2[:], wqg_sb[:, kt, CA + h * D:CA + (h + 1) * D],
                                         ahT[:, kt, :],
                                         start=(kt == 0), stop=(kt == KT_A - 1))
                    nc.scalar.activation(out=gT[:, h, :], in_=ps2[:],
                                         func=ACTF.Sigmoid, bias=0.0, scale=1.0)

            # ---- transition: hiddenT = silu(th@wsw)^T * (th@wg2)^T ----
            hidT = blk.tile([128, KT_H, 128], BF, tag="hidT")
            with tc.tile_pool(name="ps_h", bufs=3, space="PSUM") as psh:
                wsw_sb = wpool.tile([128, KT_A, NHID], BF, tag="w1536")
                nc.sync.dma_start(
                    out=wsw_sb,
                    in_=wsw_in[:].rearrange("(kt p) b m -> p kt b m", p=128)[:, :, b, :])
                swT = blk.tile([128, KT_H, 128], BF, tag="swT")
                for mt in range(KT_H):
                    ps = psh.tile([128, 128], F32, tag="ps")
                    for kt in range(KT_A):
                        nc.tensor.matmul(ps[:], wsw_sb[:, kt, bass.ts(mt, 128)],
                                         thT[:, kt, :],
                                         start=(kt == 0), stop=(kt == KT_A - 1))
                    nc.scalar.activation(out=swT[:, mt, :], in_=ps[:],
                                         func=ACTF.Silu, bias=0.0, scale=1.0)
                wg2_sb = wpool.tile([128, KT_A, NHID], BF, tag="w1536")
                nc.sync.dma_start(
                    out=wg2_sb,
                    in_=wg2_in[:].rearrange("(kt p) b m -> p kt b m", p=128)[:, :, b, :])
                for mt in range(KT_H):
                    ps = psh.tile([128, 128], F32, tag="ps")
                    for kt in range(KT_A):
                        nc.tensor.matmul(ps[:], wg2_sb[:, kt, bass.ts(mt, 128)],
                                         thT[:, kt, :],
                                         start=(kt == 0), stop=(kt == KT_A - 1))
                    g2 = att.tile([128, 128], BF, tag="g2")
                    nc.scalar.copy(out=g2, in_=ps[:])
                    nc.vector.tensor_tensor(hidT[:, mt, :], swT[:, mt, :], g2[:],
                                            OP.mult)

            # ---- attention ----
            kv_outr = kv_outb[:].rearrange("(r x) -> r x", r=NCORE)
            kT_src = kv_outr[:, 0:KV_K].rearrange(
                "r (d h j) -> d h r j", d=D, h=H)
            v_full = kvg.tile([128, NCORE, CA], BF, tag="vf")
            v_src = kv_outb[:].rearrange("(r x) -> r x", r=NCORE)[:, KV_K:] \
                .rearrange("r (j c) -> j r c", j=R)
            nc.sync.dma_start(out=v_full, in_=v_src[:])
            go_T = blk.tile([D, H, 128], BF, tag="goT")
            sums = blk.tile([R, H], F32, tag="sums")
            with tc.tile_pool(name="ps_s", bufs=2, space="PSUM") as pss, \
                 tc.tile_pool(name="ps_t", bufs=2, space="PSUM") as pst, \
                 tc.tile_pool(name="ps_o", bufs=2, space="PSUM") as pso:
                for h in range(H):
                    kT_h = att.tile([D, NCORE, 128], BF, tag="kTh")
                    nc.sync.dma_start(out=kT_h, in_=kT_src[:, h, :, :])
                    zb_t = att.tile([R, S], BF, tag="zbt")
                    nc.sync.dma_start(out=zb_t, in_=zbeta_dr[b * H + h, :, :])
                    ps_s = pss.tile([R, S], F32, tag="s")
                    for jc in range(2):
                        nc.tensor.matmul(ps_s[:, bass.ts(jc, 512)], qT[:, h, :],
                                         kT_h[:, jc * 4:(jc + 1) * 4, :],
                                         start=True, stop=False)
                        nc.tensor.matmul(ps_s[:, bass.ts(jc, 512)], ident[:],
                                         zb_t[:, bass.ts(jc, 512)],
                                         start=False, stop=True,
                                         skip_group_check=True)
                    attn = att.tile([R, S], BF, tag="attn")
                    bh_ = 1 + b * H + h
                    nc.scalar.activation(out=attn, in_=ps_s[:], func=ACTF.Exp,
                                         bias=biases[:, bh_:bh_ + 1], scale=1.0,
                                         accum_out=sums[:, h:h + 1])
                    rec = att.tile([R, 1], F32, tag="rec")
                    nc.vector.reciprocal(out=rec, in_=sums[:, h:h + 1])
                    nc.vector.tensor_scalar(attn[:], attn[:], rec[:, 0:1], None,
                                            OP.mult)
                    attnT = att.tile([128, JT, 128], BF, tag="attnT")
                    for g in range(2):
                        pt = pst.tile([128, 4, 128], BF, tag="t")
                        for q in range(4):
                            jt = g * 4 + q
                            nc.tensor.transpose(pt[:, q, :],
                                                attn[:, bass.ts(jt, 128)], ident[:])
                        nc.vector.tensor_copy(attnT[:, g * 4:(g + 1) * 4, :], pt[:])
                    ps_o = pso.tile([128, 128], F32, tag="o")
                    for jt in range(JT):
                        nc.tensor.matmul(ps_o[:D, :], v_full[:, jt, h * D:(h + 1) * D],
                                         attnT[:, jt, :],
                                         start=(jt == 0), stop=(jt == JT - 1))
                    nc.vector.tensor_tensor(go_T[:, h, :], ps_o[:D, :],
                                            gT[:, h, :], OP.mult)

            # ---- att_out = (g*o) @ wo ; b_attn = gate_attn * att_out ----
            b_attn = blk.tile([R, CA], F32, tag="batt")
            with tc.tile_pool(name="ps_wo", bufs=2, space="PSUM") as pwo:
                wo_sb = wop.tile([D, H, CA], BF, tag="wo")
                nc.sync.dma_start(
                    out=wo_sb,
                    in_=wo_in[:].rearrange("(h d) b m -> d h b m", d=D)[:, :, b, :])
                for m in range(2):
                    n0, n1 = (0, 512) if m == 0 else (512, 768)
                    ps = pwo.tile([R, 512], F32, tag="ps")
                    for h in range(H):
                        nc.tensor.matmul(ps[:, 0:n1 - n0], go_T[:, h, :],
                                         wo_sb[:, h, n0:n1],
                                         start=(h == 0), stop=(h == H - 1))
                    nc.vector.tensor_tensor(b_attn[:, n0:n1], ps[:, 0:n1 - n0],
                                            gts[:, b, n0:n1], OP.mult)

            # ---- tr = gate_tr * (hidden @ w_out); a = b_attn + tr ----
            with tc.tile_pool(name="ps_tr", bufs=2, space="PSUM") as ptr:
                wout_sb = wop.tile([128, KT_H, CA], BF, tag="wout")
                nc.sync.dma_start(
                    out=wout_sb,
                    in_=wout_in[:].rearrange("(kt p) b m -> p kt b m", p=128)[:, :, b, :])
                for m in range(2):
                    n0, n1 = (0, 512) if m == 0 else (512, 768)
                    ps = ptr.tile([R, 512], F32, tag="ps")
                    for kt in range(KT_H):
                        nc.tensor.matmul(ps[:, 0:n1 - n0], hidT[:, kt, :],
                                         wout_sb[:, kt, n0:n1],
                                         start=(kt == 0), stop=(kt == KT_H - 1))
                    tr = att.tile([R, 512], F32, tag="tr")
                    nc.vector.tensor_tensor(tr[:, 0:n1 - n0], ps[:, 0:n1 - n0],
                                            gts[:, b, CA + n0:CA + n1], OP.mult)
                    nc.vector.tensor_tensor(a_sb[:, n0:n1], b_attn[:, n0:n1],
                                            tr[:, 0:n1 - n0], OP.add)

        nc.sync.dma_start(out=a_out[:], in_=a_sb[:])

    nc.finalize()
    return nc


def _prep_inputs(a, s, z, beta, ln_s_w_attn, wg_attn, wb_attn, wq, bq, wk, wv,
                 ln_z_w, ln_z_b, wpb, wgate, wo, wsg_attn, bsg_attn,
                 ln_s_w_tr, wg_tr, wb_tr, w_swish, w_gate2, wsg_tr, bsg_tr, w_out):
    bf = ml_dtypes.bfloat16
    f32 = np.float32
    scale = 1.0 / np.sqrt(np.float32(D))

    # folded weights (shared across cores)
    wz = np.concatenate([ln_z_w[i][:, None] * wpb[i] for i in range(NB)],
                        axis=1).astype(bf)                       # [CZ, NB*H]
    bias_pb = np.concatenate([ln_z_b[i] @ wpb[i] for i in range(NB)])  # [NB*H]
    sel = np.tile(np.eye(H, dtype=np.float32), (1, NB)).astype(bf)  # [H, NB*H]
    wsn = np.stack([np.concatenate(
        [ln_s_w_attn[i][:, None] * wg_attn[i], ln_s_w_attn[i][:, None] * wb_attn[i],
         ln_s_w_tr[i][:, None] * wg_tr[i], ln_s_w_tr[i][:, None] * wb_tr[i]],
        axis=1) for i in range(NB)], axis=1).astype(bf)          # [CS, NB, 4CA]
    wsr = np.stack([np.concatenate(
        [np.concatenate([wsg_attn[i], bsg_attn[i][None, :]], 0),
         np.concatenate([wsg_tr[i], bsg_tr[i][None, :]], 0)], axis=1)
        for i in range(NB)], axis=1).astype(bf)                  # [CS+1, NB, 2CA]
    wkv = np.stack([np.concatenate([wk[i], wv[i]], 1) for i in range(NB)],
                   axis=1).astype(bf)                            # [CA, NB, 2CA]
    wqg = np.stack([np.concatenate([wq[i] * scale, wgate[i]], 1)
                    for i in range(NB)], axis=1).astype(bf)
    bqe = (bq * scale).astype(f32)                               # [NB, CA]
    wsw = np.stack([w_swish[i] for i in range(NB)], axis=1).astype(bf)
    wg2 = np.stack([w_gate2[i] for i in range(NB)], axis=1).astype(bf)
    wob = np.stack([wo[i] for i in range(NB)], axis=1).astype(bf)
    wout = np.stack([w_out[i] for i in range(NB)], axis=1).astype(bf)

    shared = dict(wz_in=np.ascontiguousarray(wz),
                  sel_in=np.ascontiguousarray(sel),
                  wsn_in=np.ascontiguousarray(wsn),
                  wsr_in=np.ascontiguousarray(wsr),
                  wkv_in=np.ascontiguousarray(wkv),
                  wqg_in=np.ascontiguousarray(wqg),
                  bq_in=np.ascontiguousarray(bqe),
                  wsw_in=np.ascontiguousarray(wsw),
                  wg2_in=np.ascontiguousarray(wg2),
                  wo_in=np.ascontiguousarray(wob),
                  wout_in=np.ascontiguousarray(wout))

    a2 = a.reshape(S, CA).astype(f32)
    s2 = s.reshape(S, CS).astype(f32)
    z2 = z.reshape(S, S, CZ).astype(bf)
    betaT = np.ascontiguousarray(
        beta.reshape(S, S, H).transpose(2, 0, 1)).astype(bf)     # [H, S, S]

    in_maps = []
    for c in range(NCORE):
        rows = slice(c * R, (c + 1) * R)
        m = dict(shared)
        m["a_in"] = np.ascontiguousarray(a2[rows])
        m["s_in"] = np.ascontiguousarray(s2[rows])
        m["z_in"] = np.ascontiguousarray(
            z2[rows].reshape(R, JT, 128, CZ).transpose(0, 2, 1, 3))
        m["betaT_in"] = np.ascontiguousarray(betaT[:, rows, :])
        in_maps.append(m)
    return in_maps, [float(x) for x in bias_pb]


_CACHE = {}


def kernel(**inputs):
    inputs = {k: np.asarray(v) for k, v in inputs.items()}
    in_maps, bias_pb = _prep_inputs(**inputs)
    key = tuple(bias_pb)
    if key not in _CACHE:
        _CACHE.clear()
        _CACHE[key] = build_program(bias_pb)
    nc = _CACHE[key]
    res = run_bass_kernel_spmd(nc, in_maps, core_ids=list(range(NCORE)),
                               trace=False)
    out = np.concatenate([res.results[c]["a_out"] for c in range(NCORE)], axis=0)
    return out.reshape(1, S, CA).astype(np.float32)


if __name__ == "__main__":
    import reference
    ins = {k: np.asarray(v) for k, v in reference.setup_inputs().items()}
    exp = np.asarray(reference.reference(**ins))
    act = kernel(**ins)
    err = np.abs(act - exp).max() / (np.abs(exp).max() + 1e-9)
    print("rel err:", err)



# revision 44
# speedup vs baseline: 6.3815x; 1.0152x over previous
"""DiffusionTransformer (AF3-style) Trainium2 kernel, 8-core SPMD.

Sharding: sequence-parallel over rows (queries). Each core owns R=128 rows of
a / z / beta. Per block, k/v are computed on local rows and AllGathered
(bf16, ~384KB per rank). zbeta (pair bias LN(z)@wpb + beta, all 4 blocks) is
precomputed once into per-core DRAM in [bh, i, j] layout.
"""
import numpy as np
import ml_dtypes
from contextlib import ExitStack

import concourse.bass as bass
import concourse.mybir as mybir
import concourse.tile as tile
from concourse import bacc
from concourse.bass_utils import run_bass_kernel_spmd
from concourse.masks import make_identity

NB, H, S, CA, CS, CZ = 4, 16, 1024, 768, 384, 128
D = CA // H            # 48
NCORE = 8
R = S // NCORE         # 128 rows per core
NHID = 2 * CA          # 1536
EPS = 1e-5
BF = mybir.dt.bfloat16
F32 = mybir.dt.float32
AX = mybir.AxisListType
OP = mybir.AluOpType
ACTF = mybir.ActivationFunctionType
JT = S // 128          # 8 j-tiles
KT_A = CA // 128       # 6 k-tiles over c_a
KT_S = CS // 128       # 3 k-tiles over c_s
KT_H = NHID // 128     # 12 k-tiles over n_hidden
KV_K = D * H * R       # 98304 flat elems of kT part
KV_TOT = KV_K + R * CA  # + v part


def _headsplit_ranges():
    """Split [0,768) hd-range at both 128-tile and 48-head boundaries.

    Returns list of (kt, p0, p1, h, d0) with kt*128+p0 == h*48+d0.
    """
    out = []
    bounds = sorted(set([x * 128 for x in range(KT_A + 1)] +
                        [h * D for h in range(H + 1)]))
    for lo, hi in zip(bounds[:-1], bounds[1:]):
        kt, p0 = lo // 128, lo % 128
        h, d0 = lo // D, lo % D
        out.append((kt, p0, hi - lo, h, d0))
    return out


HS = _headsplit_ranges()


def build_program(bias_pb):
    nc = bacc.Bacc("TRN2", target_bir_lowering=False, debug=False,
                   num_devices=NCORE)

    # ---------------- I/O ----------------
    a_in = nc.dram_tensor("a_in", [R, CA], F32, kind="ExternalInput")
    s_in = nc.dram_tensor("s_in", [R, CS], F32, kind="ExternalInput")
    z_in = nc.dram_tensor("z_in", [R, 128, JT, CZ], BF, kind="ExternalInput")
    betaT_in = nc.dram_tensor("betaT_in", [H, R, S], BF, kind="ExternalInput")
    wz_in = nc.dram_tensor("wz_in", [CZ, NB * H], BF, kind="ExternalInput")
    sel_in = nc.dram_tensor("sel_in", [H, NB * H], BF, kind="ExternalInput")
    wsn_in = nc.dram_tensor("wsn_in", [CS, NB, 4 * CA], BF, kind="ExternalInput")
    wsr_in = nc.dram_tensor("wsr_in", [CS + 1, NB, 2 * CA], BF, kind="ExternalInput")
    wkv_in = nc.dram_tensor("wkv_in", [CA, NB, 2 * CA], BF, kind="ExternalInput")
    wqg_in = nc.dram_tensor("wqg_in", [CA, NB, 2 * CA], BF, kind="ExternalInput")
    bq_in = nc.dram_tensor("bq_in", [NB, CA], F32, kind="ExternalInput")
    wsw_in = nc.dram_tensor("wsw_in", [CA, NB, NHID], BF, kind="ExternalInput")
    wg2_in = nc.dram_tensor("wg2_in", [CA, NB, NHID], BF, kind="ExternalInput")
    wo_in = nc.dram_tensor("wo_in", [CA, NB, CA], BF, kind="ExternalInput")
    wout_in = nc.dram_tensor("wout_in", [NHID, NB, CA], BF, kind="ExternalInput")
    a_out = nc.dram_tensor("a_out", [R, CA], F32, kind="ExternalOutput")

    with tile.TileContext(nc) as tc, ExitStack() as ctx:
        const = ctx.enter_context(tc.tile_pool(name="const", bufs=1))
        ident = const.tile([128, 128], BF)
        make_identity(nc, ident)
        wz_sb = const.tile([CZ, NB * H], BF)
        nc.sync.dma_start(out=wz_sb, in_=wz_in[:])
        sel_sb = const.tile([H, NB * H], BF)
        nc.sync.dma_start(out=sel_sb, in_=sel_in[:])
        biases = const.tile([128, 1 + NB * H], F32)
        nc.vector.memset(biases[:, 0:1], EPS)
        for _bh in range(NB * H):
            nc.vector.memset(biases[:, 1 + _bh:2 + _bh], float(bias_pb[_bh]))

        pers = ctx.enter_context(tc.tile_pool(name="pers", bufs=1))
        a_sb = pers.tile([R, CA], F32)
        nc.sync.dma_start(out=a_sb, in_=a_in[:])

        # internal DRAM scratch
        dram = ctx.enter_context(tc.tile_pool(name="dram", bufs=1, space="DRAM"))
        zbeta_dr = dram.tile([NB * H, R, S], BF)

        # =========== s preprocessing (once) ===========
        with tc.tile_pool(name="sprep", bufs=1) as sp:
            s_sb = sp.tile([R, CS], F32)
            nc.sync.dma_start(out=s_sb, in_=s_in[:])
            stats = sp.tile([R, 1, 6], F32)
            mv = sp.tile([R, 2], F32)
            nc.vector.bn_stats(out=stats[:, 0, :], in_=s_sb[:])
            nc.vector.bn_aggr(out=mv, in_=stats)
            rstd = sp.tile([R, 1], F32)
            nc.scalar.activation(out=rstd, in_=mv[:, 1:2], func=ACTF.Sqrt,
                                 bias=biases[:, 0:1], scale=1.0)
            nc.vector.reciprocal(out=rstd, in_=rstd)
            s_n = sp.tile([R, CS], BF)
            nc.vector.tensor_scalar(s_n[:], s_sb[:], mv[:, 0:1], rstd[:, 0:1],
                                    OP.subtract, OP.mult)
            s_b16 = sp.tile([R, CS], BF)
            nc.vector.tensor_copy(s_b16[:], s_sb[:])

            # transposed copies (persistent for all blocks)
            s_nT = pers.tile([128, KT_S, 128], BF)
            sT = pers.tile([128, KT_S, 128], BF)
            ones_row = pers.tile([1, 128], BF)
            nc.vector.memset(ones_row, 1.0)
            with tc.tile_pool(name="tp_ps", bufs=2, space="PSUM") as tps:
                for kt in range(KT_S):
                    pt = tps.tile([128, 128], BF, tag="t")
                    nc.tensor.transpose(pt[:], s_n[:, bass.ts(kt, 128)], ident[:])
                    nc.scalar.copy(out=s_nT[:, kt, :], in_=pt[:])
                    pt2 = tps.tile([128, 128], BF, tag="t")
                    nc.tensor.transpose(pt2[:], s_b16[:, bass.ts(kt, 128)], ident[:])
                    nc.scalar.copy(out=sT[:, kt, :], in_=pt2[:])

            # per-block s-derived tensors: sg/sb for attn+tr, gates attn/tr
            sgsb = pers.tile([R, NB, 4 * CA], BF)   # wg_a|wb_a|wg_t|wb_t
            gts = pers.tile([R, NB, 2 * CA], BF)    # gate_attn|gate_tr
            with tc.tile_pool(name="sw", bufs=2) as swp, \
                 tc.tile_pool(name="sps", bufs=3, space="PSUM") as sps:
                for b in range(NB):
                    wsn_sb = swp.tile([128, KT_S, 4 * CA], BF, tag="wsn")
                    nc.sync.dma_start(
                        out=wsn_sb,
                        in_=wsn_in[:].rearrange("(kt p) b m -> p kt b m", p=128)[:, :, b, :])
                    for m in range(6):  # 3072 / 512
                        ps = sps.tile([R, 512], F32, tag="ps")
                        for kt in range(KT_S):
                            nc.tensor.matmul(ps[:], s_nT[:, kt, :],
                                             wsn_sb[:, kt, bass.ts(m, 512)],
                                             start=(kt == 0), stop=(kt == KT_S - 1))
                        # cols [m*512,(m+1)*512) of [wg_a(768)|wb_a|wg_t|wb_t]
                        for lo_, hi_ in [(m * 512, m * 512 + 256), (m * 512 + 256, (m + 1) * 512)]:
                            mat = lo_ // CA  # 0..3
                            f = ACTF.Sigmoid if mat in (0, 2) else ACTF.Copy
                            nc.scalar.activation(
                                out=sgsb[:, b, lo_:hi_], in_=ps[:, lo_ - m * 512:hi_ - m * 512],
                                func=f, bias=0.0 if f == ACTF.Copy else 0.0, scale=1.0)
                    wsr_sb = swp.tile([128, KT_S, 2 * CA], BF, tag="wsr")
                    nc.sync.dma_start(
                        out=wsr_sb,
                        in_=wsr_in[:CS].rearrange("(kt p) b m -> p kt b m", p=128)[:, :, b, :])
                    wsr_last = swp.tile([1, 2 * CA], BF, tag="wsrl")
                    nc.sync.dma_start(out=wsr_last, in_=wsr_in[CS:CS + 1, b, :])
                    for m in range(3):  # 1536 / 512
                        ps = sps.tile([R, 512], F32, tag="ps")
                        for kt in range(KT_S):
                            nc.tensor.matmul(ps[:], sT[:, kt, :],
                                             wsr_sb[:, kt, bass.ts(m, 512)],
                                             start=(kt == 0), stop=False)
                        nc.tensor.matmul(ps[:], ones_row[:],
                                         wsr_last[:, bass.ts(m, 512)],
                                         start=False, stop=True)
                        nc.scalar.activation(out=gts[:, b, bass.ts(m, 512)],
                                             in_=ps[:], func=ACTF.Sigmoid,
                                             bias=0.0, scale=1.0)

        # =========== z preprocessing (once) ===========
        with tc.tile_pool(name="zslab", bufs=4) as zsl, \
             tc.tile_pool(name="zsm", bufs=4) as zsm, \
             tc.tile_pool(name="ztp", bufs=3, space="PSUM") as ztp, \
             tc.tile_pool(name="zbp", bufs=2, space="PSUM") as zbp:
            for i in range(R):
                zt = zsl.tile([128, JT, CZ], BF, tag="z")
                nc.sync.dma_start(out=zt, in_=z_in[i])
                bsel = zsl.tile([H, S], BF, tag="bsel")
                nc.sync.dma_start(out=bsel, in_=betaT_in[:, i, :])
                st8 = zsm.tile([128, JT, 6], F32, tag="st")
                for jt in range(JT):
                    nc.vector.bn_stats(out=st8[:, jt, :], in_=zt[:, jt, :])
                # pooled even/odd moments, vectorized over all 8 j-tiles:
                # mean = (m_e+m_o)/2; var = (64v_e+64v_o)/128 + ((m_e-m_o)/2)^2
                mrow = zsm.tile([128, JT], F32, tag="mrow")
                nc.gpsimd.tensor_tensor(mrow[:], st8[:, :, 1], st8[:, :, 4], OP.add)
                nc.vector.tensor_scalar(mrow[:], mrow[:], 0.5, None, OP.mult)
                dm = zsm.tile([128, JT], F32, tag="dm")
                nc.gpsimd.tensor_tensor(dm[:], st8[:, :, 1], st8[:, :, 4],
                                        OP.subtract)
                nc.gpsimd.tensor_tensor(dm[:], dm[:], dm[:], OP.mult)
                nc.vector.tensor_scalar(dm[:], dm[:], 0.25, None, OP.mult)
                var = zsm.tile([128, JT], F32, tag="var")
                nc.gpsimd.tensor_tensor(var[:], st8[:, :, 2], st8[:, :, 5], OP.add)
                nc.vector.tensor_scalar(var[:], var[:], 1.0 / CZ, None, OP.mult)
                nc.vector.tensor_tensor(var[:], var[:], dm[:], OP.add)
                rst = zsm.tile([128, JT], F32, tag="rst")
                nc.scalar.activation(out=rst, in_=var[:], func=ACTF.Sqrt,
                                     bias=biases[:, 0:1], scale=1.0)
                nc.vector.reciprocal(out=rst, in_=rst)
                zh = zsm.tile([128, JT, CZ], BF, tag="zh")
                for jt in range(JT):
                    nc.vector.tensor_scalar(zh[:, jt, :], zt[:, jt, :],
                                            mrow[:, jt:jt + 1], rst[:, jt:jt + 1],
                                            OP.subtract, OP.mult)
                zhT = zsm.tile([128, JT, 128], BF, tag="zhT")
                for g in range(2):  # transpose 8 tiles, copy in 2 batches
                    pt = ztp.tile([128, 4, 128], BF, tag="t")
                    for q in range(4):
                        jt = g * 4 + q
                        nc.tensor.transpose(pt[:, q, :], zh[:, jt, :], ident[:])
                    nc.scalar.copy(out=zhT[:, g * 4:(g + 1) * 4, :], in_=pt[:])
                zb = zbp.tile([NB * H, S], F32, tag="zb")
                for jc in range(2):
                    nc.tensor.matmul(zb[:, bass.ts(jc, 512)], wz_sb[:],
                                     zhT[:].rearrange("p jt j -> p (jt j)")[:, bass.ts(jc, 512)],
                                     start=True, stop=False)
                    nc.tensor.matmul(zb[:, bass.ts(jc, 512)], sel_sb[:],
                                     bsel[:, bass.ts(jc, 512)],
                                     start=False, stop=True)
                zbs = zsm.tile([NB * H, S], BF, tag="zbs")
                nc.scalar.copy(out=zbs, in_=zb[:])
                nc.sync.dma_start(out=zbeta_dr[:, i, :], in_=zbs)

        # =========== block loop ===========
        wpool = ctx.enter_context(tc.tile_pool(name="wpool", bufs=2))
        wop = ctx.enter_context(tc.tile_pool(name="wop", bufs=1))
        blk = ctx.enter_context(tc.tile_pool(name="blk", bufs=1))
        kvg = ctx.enter_context(tc.tile_pool(name="kvg", bufs=1))
        att = ctx.enter_context(tc.tile_pool(name="att", bufs=3))
        dramc = ctx.enter_context(tc.tile_pool(name="dramc", bufs=2, space="DRAM"))

        for b in range(NB):
            # ---- ada_ln(a) shared stats ----
            with tc.tile_pool(name="lnp", bufs=1) as lnp, \
                 tc.tile_pool(name="ps_ln", bufs=2, space="PSUM") as pln:
                st3 = lnp.tile([R, 3, 6], F32)
                for g_ in range(3):
                    nc.vector.bn_stats(out=st3[:, g_, :],
                                       in_=a_sb[:, bass.ts(g_, 256)])
                mv = lnp.tile([R, 2], F32)
                nc.vector.bn_aggr(out=mv, in_=st3)
                rstd = lnp.tile([R, 1], F32)
                nc.scalar.activation(out=rstd, in_=mv[:, 1:2], func=ACTF.Sqrt,
                                     bias=biases[:, 0:1], scale=1.0)
                nc.vector.reciprocal(out=rstd, in_=rstd)
                xhat = lnp.tile([R, CA], BF)
                nc.vector.tensor_scalar(xhat[:], a_sb[:], mv[:, 0:1], rstd[:, 0:1],
                                        OP.subtract, OP.mult)
                ah = blk.tile([R, CA], BF, tag="ah")
                nc.vector.tensor_tensor(ah[:], xhat[:], sgsb[:, b, 0:CA], OP.mult)
                nc.vector.tensor_tensor(ah[:], ah[:], sgsb[:, b, CA:2 * CA], OP.add)
                th = blk.tile([R, CA], BF, tag="th")
                nc.vector.tensor_tensor(th[:], xhat[:], sgsb[:, b, 2 * CA:3 * CA], OP.mult)
                nc.vector.tensor_tensor(th[:], th[:], sgsb[:, b, 3 * CA:4 * CA], OP.add)
                ahT = blk.tile([128, KT_A, 128], BF, tag="ahT")
                for kt in range(KT_A):
                    pt = pln.tile([128, 128], BF, tag="t")
                    nc.tensor.transpose(pt[:], ah[:, bass.ts(kt, 128)], ident[:])
                    nc.scalar.copy(out=ahT[:, kt, :], in_=pt[:])
                thT = blk.tile([128, KT_A, 128], BF, tag="thT")
                for kt in range(KT_A):
                    pt = pln.tile([128, 128], BF, tag="t")
                    nc.tensor.transpose(pt[:], th[:, bass.ts(kt, 128)], ident[:])
                    nc.scalar.copy(out=thT[:, kt, :], in_=pt[:])

            # ---- kv local + gather ----
            kv_inb = dramc.tile([KV_TOT], BF, tag="kvin")
            kv_outb = dramc.tile([NCORE * KV_TOT], BF, tag="kvout",
                                 addr_space="Shared")
            with tc.tile_pool(name="ps_kv", bufs=3, space="PSUM") as pkv:
                wkv_sb = wpool.tile([128, KT_A, 2 * CA], BF, tag="w1536")
                nc.sync.dma_start(
                    out=wkv_sb,
                    in_=wkv_in[:].rearrange("(kt p) b m -> p kt b m", p=128)[:, :, b, :])
                kv_sb = kvg.tile([R, 2 * CA], BF, tag="kv")
                for m in range(3):  # 1536/512
                    ps = pkv.tile([R, 512], F32, tag="ps")
                    for kt in range(KT_A):
                        nc.tensor.matmul(ps[:], ahT[:, kt, :],
                                         wkv_sb[:, kt, bass.ts(m, 512)],
                                         start=(kt == 0), stop=(kt == KT_A - 1))
                    nc.scalar.copy(out=kv_sb[:, bass.ts(m, 512)], in_=ps[:])
                # kT head-tiles
                kT_loc = kvg.tile([D, H, 128], BF, tag="kT")
                for h in range(H):
                    pt = pkv.tile([128, 128], BF, tag="t")
                    nc.tensor.transpose(pt[:D, :], kv_sb[:, h * D:(h + 1) * D], ident[:])
                    nc.vector.tensor_copy(kT_loc[:, h, :], pt[:D, :])
                nc.sync.dma_start(out=kv_inb[0:KV_K].rearrange("(d x) -> d x", d=D),
                                  in_=kT_loc[:].rearrange("d h j -> d (h j)"))
                nc.sync.dma_start(out=kv_inb[KV_K:].rearrange("(j c) -> j c", j=R),
                                  in_=kv_sb[:, CA:])
            nc.gpsimd.collective_compute(
                "AllGather", OP.bypass,
                replica_groups=[list(range(NCORE))],
                ins=[kv_inb[:].opt()], outs=[kv_outb[:].opt()])

            # ---- qT, gT (transposed head layout) ----
            with tc.tile_pool(name="ps_qg", bufs=3, space="PSUM") as pqg:
                wqg_sb = wpool.tile([128, KT_A, 2 * CA], BF, tag="w1536")
                nc.sync.dma_start(
                    out=wqg_sb,
                    in_=wqg_in[:].rearrange("(kt p) b m -> p kt b m", p=128)[:, :, b, :])
                bq_sb = blk.tile([D, H], F32, tag="bq")
                nc.sync.dma_start(
                    out=bq_sb, in_=bq_in[b].rearrange("(h d) -> d h", d=D))
                qT = blk.tile([D, H, 128], BF, tag="qT")
                gT = blk.tile([D, H, 128], BF, tag="gT")
                for h in range(H):
                    ps = pqg.tile([D, 128], F32, tag="ps")
                    for kt in range(KT_A):
                        nc.tensor.matmul(ps[:], wqg_sb[:, kt, h * D:(h + 1) * D],
                                         ahT[:, kt, :],
                                         start=(kt == 0), stop=(kt == KT_A - 1))
                    nc.scalar.activation(out=qT[:, h, :], in_=ps[:],
                                         func=ACTF.Identity,
                                         bias=bq_sb[:, h:h + 1], scale=1.0)
                    ps2 = pqg.tile([D, 128], F32, tag="ps")
                    for kt in range(KT_A):
                        nc.tensor.matmul(ps2[:], wqg_sb[:, kt, CA + h * D:CA + (h + 1) * D],
                                         ahT[:, kt, :],
                                         start=(kt == 0), stop=(kt == KT_A - 1))
                    nc.scalar.activation(out=gT[:, h, :], in_=ps2[:],
                                         func=ACTF.Sigmoid, bias=0.0, scale=1.0)

            # ---- transition: hiddenT = silu(th@wsw)^T * (th@wg2)^T ----
            hidT = blk.tile([128, KT_H, 128], BF, tag="hidT")
            with tc.tile_pool(name="ps_h", bufs=3, space="PSUM") as psh:
                wsw_sb = wpool.tile([128, KT_A, NHID], BF, tag="w1536")
                nc.sync.dma_start(
                    out=wsw_sb,
                    in_=wsw_in[:].rearrange("(kt p) b m -> p kt b m", p=128)[:, :, b, :])
                swT = blk.tile([128, KT_H, 128], BF, tag="swT")
                for mt in range(KT_H):
                    ps = psh.tile([128, 128], F32, tag="ps")
                    for kt in range(KT_A):
                        nc.tensor.matmul(ps[:], wsw_sb[:, kt, bass.ts(mt, 128)],
                                         thT[:, kt, :],
                                         start=(kt == 0), stop=(kt == KT_A - 1))
                    nc.scalar.activation(out=swT[:, mt, :], in_=ps[:],
                                         func=ACTF.Silu, bias=0.0, scale=1.0)
                wg2_sb = wpool.tile([128, KT_A, NHID], BF, tag="w1536")
                nc.sync.dma_start(
                    out=wg2_sb,
                    in_=wg2_in[:].rearrange("(kt p) b m -> p kt b m", p=128)[:, :, b, :])
                for mt in range(KT_H):
                    ps = psh.tile([128, 128], F32, tag="ps")
                    for kt in range(KT_A):
                        nc.tensor.matmul(ps[:], wg2_sb[:, kt, bass.ts(mt, 128)],
                                         thT[:, kt, :],
                                         start=(kt == 0), stop=(kt == KT_A - 1))
                    g2 = att.tile([128, 128], BF, tag="g2")
                    nc.scalar.copy(out=g2, in_=ps[:])
                    nc.vector.tensor_tensor(hidT[:, mt, :], swT[:, mt, :], g2[:],
                                            OP.mult)

            # ---- attention ----
            kv_outr = kv_outb[:].rearrange("(r x) -> r x", r=NCORE)
            kT_src = kv_outr[:, 0:KV_K].rearrange(
                "r (d h j) -> d h r j", d=D, h=H)
            v_full = kvg.tile([128, NCORE, CA], BF, tag="vf")
            v_src = kv_outb[:].rearrange("(r x) -> r x", r=NCORE)[:, KV_K:] \
                .rearrange("r (j c) -> j r c", j=R)
            nc.sync.dma_start(out=v_full, in_=v_src[:])
            go_T = blk.tile([D, H, 128], BF, tag="goT")
            sums = blk.tile([R, H], F32, tag="sums")
            with tc.tile_pool(name="ps_s", bufs=2, space="PSUM") as pss, \
                 tc.tile_pool(name="ps_t", bufs=2, space="PSUM") as pst, \
                 tc.tile_pool(name="ps_o", bufs=2, space="PSUM") as pso:
                for h in range(H):
                    kT_h = att.tile([D, NCORE, 128], BF, tag="kTh")
                    nc.sync.dma_start(out=kT_h, in_=kT_src[:, h, :, :])
                    zb_t = att.tile([R, S], BF, tag="zbt")
                    nc.sync.dma_start(out=zb_t, in_=zbeta_dr[b * H + h, :, :])
                    ps_s = pss.tile([R, S], F32, tag="s")
                    for jc in range(2):
                        nc.tensor.matmul(ps_s[:, bass.ts(jc, 512)], qT[:, h, :],
                                         kT_h[:, jc * 4:(jc + 1) * 4, :],
                                         start=True, stop=False)
                        nc.tensor.matmul(ps_s[:, bass.ts(jc, 512)], ident[:],
                                         zb_t[:, bass.ts(jc, 512)],
                                         start=False, stop=True,
                                         skip_group_check=True)
                    attn = att.tile([R, S], BF, tag="attn")
                    bh_ = 1 + b * H + h
                    nc.scalar.activation(out=attn, in_=ps_s[:], func=ACTF.Exp,
                                         bias=biases[:, bh_:bh_ + 1], scale=1.0,
                                         accum_out=sums[:, h:h + 1])
                    rec = att.tile([R, 1], F32, tag="rec")
                    nc.vector.reciprocal(out=rec, in_=sums[:, h:h + 1])
                    nc.vector.tensor_scalar(attn[:], attn[:], rec[:, 0:1], None,
                                            OP.mult)
                    attnT = att.tile([128, JT, 128], BF, tag="attnT")
                    for g in range(2):
                        pt = pst.tile([128, 4, 128], BF, tag="t")
                        for q in range(4):
                            jt = g * 4 + q
                            nc.tensor.transpose(pt[:, q, :],
                                                attn[:, bass.ts(jt, 128)], ident[:])
                        nc.vector.tensor_copy(attnT[:, g * 4:(g + 1) * 4, :], pt[:])
                    ps_o = pso.tile([128, 128], F32, tag="o")
                    for jt in range(JT):
                        nc.tensor.matmul(ps_o[:D, :], v_full[:, jt, h * D:(h + 1) * D],
                                         attnT[:, jt, :],
                                         start=(jt == 0), stop=(jt == JT - 1))
                    nc.vector.tensor_tensor(go_T[:, h, :], ps_o[:D, :],
                                            gT[:, h, :], OP.mult)

            # ---- att_out = (g*o) @ wo ; b_attn = gate_attn * att_out ----
            b_attn = blk.tile([R, CA], F32, tag="batt")
            with tc.tile_pool(name="ps_wo", bufs=2, space="PSUM") as pwo:
                wo_sb = wop.tile([D, H, CA], BF, tag="wo")
                nc.sync.dma_start(
                    out=wo_sb,
                    in_=wo_in[:].rearrange("(h d) b m -> d h b m", d=D)[:, :, b, :])
                for m in range(2):
                    n0, n1 = (0, 512) if m == 0 else (512, 768)
                    ps = pwo.tile([R, 512], F32, tag="ps")
                    for h in range(H):
                        nc.tensor.matmul(ps[:, 0:n1 - n0], go_T[:, h, :],
                                         wo_sb[:, h, n0:n1],
                                         start=(h == 0), stop=(h == H - 1))
                    nc.vector.tensor_tensor(b_attn[:, n0:n1], ps[:, 0:n1 - n0],
                                            gts[:, b, n0:n1], OP.mult)

            # ---- tr = gate_tr * (hidden @ w_out); a = b_attn + tr ----
            with tc.tile_pool(name="ps_tr", bufs=2, space="PSUM") as ptr:
                wout_sb = wop.tile([128, KT_H, CA], BF, tag="wout")
                nc.sync.dma_start(
                    out=wout_sb,
                    in_=wout_in[:].rearrange("(kt p) b m -> p kt b m", p=128)[:, :, b, :])
                for m in range(2):
                    n0, n1 = (0, 512) if m == 0 else (512, 768)
                    ps = ptr.tile([R, 512], F32, tag="ps")
                    for kt in range(KT_H):
                        nc.tensor.matmul(ps[:, 0:n1 - n0], hidT[:, kt, :],
                                         wout_sb[:, kt, n0:n1],
                                         start=(kt == 0), stop=(kt == KT_H - 1))
                    tr = att.tile([R, 512], F32, tag="tr")
                    nc.vector.tensor_tensor(tr[:, 0:n1 - n0], ps[:, 0:n1 - n0],
                                            gts[:, b, CA + n0:CA + n1], OP.mult)
                    nc.vector.tensor_tensor(a_sb[:, n0:n1], b_attn[:, n0:n1],
                                            tr[:, 0:n1 - n0], OP.add)

        nc.sync.dma_start(out=a_out[:], in_=a_sb[:])

    nc.finalize()
    return nc


def _prep_inputs(a, s, z, beta, ln_s_w_attn, wg_attn, wb_attn, wq, bq, wk, wv,
                 ln_z_w, ln_z_b, wpb, wgate, wo, wsg_attn, bsg_attn,
                 ln_s_w_tr, wg_tr, wb_tr, w_swish, w_gate2, wsg_tr, bsg_tr, w_out):
    bf = ml_dtypes.bfloat16
    f32 = np.float32
    scale = 1.0 / np.sqrt(np.float32(D))

    # folded weights (shared across cores)
    wz = np.concatenate([ln_z_w[i][:, None] * wpb[i] for i in range(NB)],
                        axis=1).astype(bf)                       # [CZ, NB*H]
    bias_pb = np.concatenate([ln_z_b[i] @ wpb[i] for i in range(NB)])  # [NB*H]
    sel = np.tile(np.eye(H, dtype=np.float32), (1, NB)).astype(bf)  # [H, NB*H]
    wsn = np.stack([np.concatenate(
        [ln_s_w_attn[i][:, None] * wg_attn[i], ln_s_w_attn[i][:, None] * wb_attn[i],
         ln_s_w_tr[i][:, None] * wg_tr[i], ln_s_w_tr[i][:, None] * wb_tr[i]],
        axis=1) for i in range(NB)], axis=1).astype(bf)          # [CS, NB, 4CA]
    wsr = np.stack([np.concatenate(
        [np.concatenate([wsg_attn[i], bsg_attn[i][None, :]], 0),
         np.concatenate([wsg_tr[i], bsg_tr[i][None, :]], 0)], axis=1)
        for i in range(NB)], axis=1).astype(bf)                  # [CS+1, NB, 2CA]
    wkv = np.stack([np.concatenate([wk[i], wv[i]], 1) for i in range(NB)],
                   axis=1).astype(bf)                            # [CA, NB, 2CA]
    wqg = np.stack([np.concatenate([wq[i] * scale, wgate[i]], 1)
                    for i in range(NB)], axis=1).astype(bf)
    bqe = (bq * scale).astype(f32)                               # [NB, CA]
    wsw = np.stack([w_swish[i] for i in range(NB)], axis=1).astype(bf)
    wg2 = np.stack([w_gate2[i] for i in range(NB)], axis=1).astype(bf)
    wob = np.stack([wo[i] for i in range(NB)], axis=1).astype(bf)
    wout = np.stack([w_out[i] for i in range(NB)], axis=1).astype(bf)

    shared = dict(wz_in=np.ascontiguousarray(wz),
                  sel_in=np.ascontiguousarray(sel),
                  wsn_in=np.ascontiguousarray(wsn),
                  wsr_in=np.ascontiguousarray(wsr),
                  wkv_in=np.ascontiguousarray(wkv),
                  wqg_in=np.ascontiguousarray(wqg),
                  bq_in=np.ascontiguousarray(bqe),
                  wsw_in=np.ascontiguousarray(wsw),
                  wg2_in=np.ascontiguousarray(wg2),
                  wo_in=np.ascontiguousarray(wob),
                  wout_in=np.ascontiguousarray(wout))

    a2 = a.reshape(S, CA).astype(f32)
    s2 = s.reshape(S, CS).astype(f32)
    z2 = z.reshape(S, S, CZ).astype(bf)
    betaT = np.ascontiguousarray(
        beta.reshape(S, S, H).transpose(2, 0, 1)).astype(bf)     # [H, S, S]

    in_maps = []
    for c in range(NCORE):
        rows = slice(c * R, (c + 1) * R)
        m = dict(shared)
        m["a_in"] = np.ascontiguousarray(a2[rows])
        m["s_in"] = np.ascontiguousarray(s2[rows])
        m["z_in"] = np.ascontiguousarray(
            z2[rows].reshape(R, JT, 128, CZ).transpose(0, 2, 1, 3))
        m["betaT_in"] = np.ascontiguousarray(betaT[:, rows, :])
        in_maps.append(m)
    return in_maps, [float(x) for x in bias_pb]


_CACHE = {}


def kernel(**inputs):
    inputs = {k: np.asarray(v) for k, v in inputs.items()}
    in_maps, bias_pb = _prep_inputs(**inputs)
    key = tuple(bias_pb)
    if key not in _CACHE:
        _CACHE.clear()
        _CACHE[key] = build_program(bias_pb)
    nc = _CACHE[key]
    res = run_bass_kernel_spmd(nc, in_maps, core_ids=list(range(NCORE)),
                               trace=False)
    out = np.concatenate([res.results[c]["a_out"] for c in range(NCORE)], axis=0)
    return out.reshape(1, S, CA).astype(np.float32)


if __name__ == "__main__":
    import reference
    ins = {k: np.asarray(v) for k, v in reference.setup_inputs().items()}
    exp = np.asarray(reference.reference(**ins))
    act = kernel(**ins)
    err = np.abs(act - exp).max() / (np.abs(exp).max() + 1e-9)
    print("rel err:", err)



# revision 49
# speedup vs baseline: 6.8963x; 1.0807x over previous
"""DiffusionTransformer (AF3-style) Trainium2 kernel, 8-core SPMD.

Sharding: sequence-parallel over rows (queries). Each core owns R=128 rows of
a / z / beta. Per block, k/v are computed on local rows and AllGathered
(bf16, ~384KB per rank). zbeta (pair bias LN(z)@wpb + beta, all 4 blocks) is
precomputed once into per-core DRAM in [bh, i, j] layout.
"""
import numpy as np
import ml_dtypes
from contextlib import ExitStack

import concourse.bass as bass
import concourse.mybir as mybir
import concourse.tile as tile
from concourse import bacc
from concourse.bass_utils import run_bass_kernel_spmd
from concourse.masks import make_identity

NB, H, S, CA, CS, CZ = 4, 16, 1024, 768, 384, 128
D = CA // H            # 48
NCORE = 8
R = S // NCORE         # 128 rows per core
NHID = 2 * CA          # 1536
EPS = 1e-5
BF = mybir.dt.bfloat16
F32 = mybir.dt.float32
AX = mybir.AxisListType
OP = mybir.AluOpType
ACTF = mybir.ActivationFunctionType
JT = S // 128          # 8 j-tiles
KT_A = CA // 128       # 6 k-tiles over c_a
KT_S = CS // 128       # 3 k-tiles over c_s
KT_H = NHID // 128     # 12 k-tiles over n_hidden
KV_K = D * H * R       # 98304 flat elems of kT part
KV_TOT = KV_K + R * CA  # + v part


def _headsplit_ranges():
    """Split [0,768) hd-range at both 128-tile and 48-head boundaries.

    Returns list of (kt, p0, p1, h, d0) with kt*128+p0 == h*48+d0.
    """
    out = []
    bounds = sorted(set([x * 128 for x in range(KT_A + 1)] +
                        [h * D for h in range(H + 1)]))
    for lo, hi in zip(bounds[:-1], bounds[1:]):
        kt, p0 = lo // 128, lo % 128
        h, d0 = lo // D, lo % D
        out.append((kt, p0, hi - lo, h, d0))
    return out


HS = _headsplit_ranges()


def build_program(bias_pb):
    nc = bacc.Bacc("TRN2", target_bir_lowering=False, debug=False,
                   num_devices=NCORE)

    # ---------------- I/O ----------------
    a_in = nc.dram_tensor("a_in", [R, CA], F32, kind="ExternalInput")
    s_in = nc.dram_tensor("s_in", [R, CS], F32, kind="ExternalInput")
    z_in = nc.dram_tensor("z_in", [R, 128, JT, CZ], BF, kind="ExternalInput")
    betaT_in = nc.dram_tensor("betaT_in", [H, R, S], BF, kind="ExternalInput")
    wz_in = nc.dram_tensor("wz_in", [CZ, NB * H], BF, kind="ExternalInput")
    sel_in = nc.dram_tensor("sel_in", [H, NB * H], BF, kind="ExternalInput")
    wsn_in = nc.dram_tensor("wsn_in", [CS, NB, 4 * CA], BF, kind="ExternalInput")
    wsr_in = nc.dram_tensor("wsr_in", [CS + 1, NB, 2 * CA], BF, kind="ExternalInput")
    wkv_in = nc.dram_tensor("wkv_in", [CA, NB, 2 * CA], BF, kind="ExternalInput")
    wqg_in = nc.dram_tensor("wqg_in", [CA, NB, 2 * CA], BF, kind="ExternalInput")
    bq_in = nc.dram_tensor("bq_in", [NB, CA], F32, kind="ExternalInput")
    wsw_in = nc.dram_tensor("wsw_in", [CA, NB, NHID], BF, kind="ExternalInput")
    wg2_in = nc.dram_tensor("wg2_in", [CA, NB, NHID], BF, kind="ExternalInput")
    wo_in = nc.dram_tensor("wo_in", [CA, NB, CA], BF, kind="ExternalInput")
    wout_in = nc.dram_tensor("wout_in", [NHID, NB, CA], BF, kind="ExternalInput")
    a_out = nc.dram_tensor("a_out", [R, CA], F32, kind="ExternalOutput")

    with tile.TileContext(nc) as tc, ExitStack() as ctx:
        const = ctx.enter_context(tc.tile_pool(name="const", bufs=1))
        ident = const.tile([128, 128], BF)
        make_identity(nc, ident)
        wz_sb = const.tile([CZ, NB * H], BF)
        nc.sync.dma_start(out=wz_sb, in_=wz_in[:])
        sel_sb = const.tile([H, NB * H], BF)
        nc.sync.dma_start(out=sel_sb, in_=sel_in[:])
        biases = const.tile([128, 1 + NB * H], F32)
        nc.vector.memset(biases[:, 0:1], EPS)
        for _bh in range(NB * H):
            nc.vector.memset(biases[:, 1 + _bh:2 + _bh], float(bias_pb[_bh]))

        pers = ctx.enter_context(tc.tile_pool(name="pers", bufs=1))
        a_sb = pers.tile([R, CA], F32)
        nc.sync.dma_start(out=a_sb, in_=a_in[:])

        # internal DRAM scratch
        dram = ctx.enter_context(tc.tile_pool(name="dram", bufs=1, space="DRAM"))
        zbeta_dr = dram.tile([NB * H, R, S], BF)

        # =========== s preprocessing (once) ===========
        with tc.tile_pool(name="sprep", bufs=1) as sp:
            s_sb = sp.tile([R, CS], F32)
            nc.sync.dma_start(out=s_sb, in_=s_in[:])
            stats = sp.tile([R, 1, 6], F32)
            mv = sp.tile([R, 2], F32)
            nc.vector.bn_stats(out=stats[:, 0, :], in_=s_sb[:])
            nc.vector.bn_aggr(out=mv, in_=stats)
            rstd = sp.tile([R, 1], F32)
            nc.scalar.activation(out=rstd, in_=mv[:, 1:2], func=ACTF.Sqrt,
                                 bias=biases[:, 0:1], scale=1.0)
            nc.vector.reciprocal(out=rstd, in_=rstd)
            s_n = sp.tile([R, CS], BF)
            nc.vector.tensor_scalar(s_n[:], s_sb[:], mv[:, 0:1], rstd[:, 0:1],
                                    OP.subtract, OP.mult)
            s_b16 = sp.tile([R, CS], BF)
            nc.vector.tensor_copy(s_b16[:], s_sb[:])

            # transposed copies (persistent for all blocks)
            s_nT = pers.tile([128, KT_S, 128], BF)
            sT = pers.tile([128, KT_S, 128], BF)
            ones_row = pers.tile([1, 128], BF)
            nc.vector.memset(ones_row, 1.0)
            with tc.tile_pool(name="tp_ps", bufs=2, space="PSUM") as tps:
                for kt in range(KT_S):
                    pt = tps.tile([128, 128], BF, tag="t")
                    nc.tensor.transpose(pt[:], s_n[:, bass.ts(kt, 128)], ident[:])
                    nc.scalar.copy(out=s_nT[:, kt, :], in_=pt[:])
                    pt2 = tps.tile([128, 128], BF, tag="t")
                    nc.tensor.transpose(pt2[:], s_b16[:, bass.ts(kt, 128)], ident[:])
                    nc.scalar.copy(out=sT[:, kt, :], in_=pt2[:])

            # per-block s-derived tensors: sg/sb for attn+tr, gates attn/tr
            sgsb = pers.tile([R, NB, 4 * CA], BF)   # wg_a|wb_a|wg_t|wb_t
            gts = pers.tile([R, NB, 2 * CA], BF)    # gate_attn|gate_tr
            with tc.tile_pool(name="sw", bufs=2) as swp, \
                 tc.tile_pool(name="sps", bufs=3, space="PSUM") as sps:
                for b in range(NB):
                    wsn_sb = swp.tile([128, KT_S, 4 * CA], BF, tag="wsn")
                    nc.sync.dma_start(
                        out=wsn_sb,
                        in_=wsn_in[:].rearrange("(kt p) b m -> p kt b m", p=128)[:, :, b, :])
                    for m in range(6):  # 3072 / 512
                        ps = sps.tile([R, 512], F32, tag="ps")
                        for kt in range(KT_S):
                            nc.tensor.matmul(ps[:], s_nT[:, kt, :],
                                             wsn_sb[:, kt, bass.ts(m, 512)],
                                             start=(kt == 0), stop=(kt == KT_S - 1))
                        # cols [m*512,(m+1)*512) of [wg_a(768)|wb_a|wg_t|wb_t]
                        for lo_, hi_ in [(m * 512, m * 512 + 256), (m * 512 + 256, (m + 1) * 512)]:
                            mat = lo_ // CA  # 0..3
                            f = ACTF.Sigmoid if mat in (0, 2) else ACTF.Copy
                            nc.scalar.activation(
                                out=sgsb[:, b, lo_:hi_], in_=ps[:, lo_ - m * 512:hi_ - m * 512],
                                func=f, bias=0.0 if f == ACTF.Copy else 0.0, scale=1.0)
                    wsr_sb = swp.tile([128, KT_S, 2 * CA], BF, tag="wsr")
                    nc.sync.dma_start(
                        out=wsr_sb,
                        in_=wsr_in[:CS].rearrange("(kt p) b m -> p kt b m", p=128)[:, :, b, :])
                    wsr_last = swp.tile([1, 2 * CA], BF, tag="wsrl")
                    nc.sync.dma_start(out=wsr_last, in_=wsr_in[CS:CS + 1, b, :])
                    for m in range(3):  # 1536 / 512
                        ps = sps.tile([R, 512], F32, tag="ps")
                        for kt in range(KT_S):
                            nc.tensor.matmul(ps[:], sT[:, kt, :],
                                             wsr_sb[:, kt, bass.ts(m, 512)],
                                             start=(kt == 0), stop=False)
                        nc.tensor.matmul(ps[:], ones_row[:],
                                         wsr_last[:, bass.ts(m, 512)],
                                         start=False, stop=True)
                        nc.scalar.activation(out=gts[:, b, bass.ts(m, 512)],
                                             in_=ps[:], func=ACTF.Sigmoid,
                                             bias=0.0, scale=1.0)

        # =========== z preprocessing (once) ===========
        with tc.tile_pool(name="zslab", bufs=4) as zsl, \
             tc.tile_pool(name="zsm", bufs=4) as zsm, \
             tc.tile_pool(name="ztp", bufs=3, space="PSUM") as ztp, \
             tc.tile_pool(name="zbp", bufs=2, space="PSUM") as zbp:
            for i in range(R):
                zt = zsl.tile([128, JT, CZ], BF, tag="z")
                nc.sync.dma_start(out=zt, in_=z_in[i])
                bsel = zsl.tile([H, S], BF, tag="bsel")
                nc.sync.dma_start(out=bsel, in_=betaT_in[:, i, :])
                st8 = zsm.tile([128, JT, 6], F32, tag="st")
                for jt in range(JT):
                    nc.vector.bn_stats(out=st8[:, jt, :], in_=zt[:, jt, :])
                # pooled even/odd moments, vectorized over all 8 j-tiles:
                # mean = (m_e+m_o)/2; var = (64v_e+64v_o)/128 + ((m_e-m_o)/2)^2
                mrow = zsm.tile([128, JT], F32, tag="mrow")
                nc.gpsimd.tensor_tensor(mrow[:], st8[:, :, 1], st8[:, :, 4], OP.add)
                nc.vector.tensor_scalar(mrow[:], mrow[:], 0.5, None, OP.mult)
                dm = zsm.tile([128, JT], F32, tag="dm")
                nc.gpsimd.tensor_tensor(dm[:], st8[:, :, 1], st8[:, :, 4],
                                        OP.subtract)
                nc.gpsimd.tensor_tensor(dm[:], dm[:], dm[:], OP.mult)
                nc.vector.tensor_scalar(dm[:], dm[:], 0.25, None, OP.mult)
                var = zsm.tile([128, JT], F32, tag="var")
                nc.gpsimd.tensor_tensor(var[:], st8[:, :, 2], st8[:, :, 5], OP.add)
                nc.vector.tensor_scalar(var[:], var[:], 1.0 / CZ, None, OP.mult)
                nc.vector.tensor_tensor(var[:], var[:], dm[:], OP.add)
                rst = zsm.tile([128, JT], F32, tag="rst")
                nc.scalar.activation(out=rst, in_=var[:], func=ACTF.Sqrt,
                                     bias=biases[:, 0:1], scale=1.0)
                nc.vector.reciprocal(out=rst, in_=rst)
                zh = zsm.tile([128, JT, CZ], BF, tag="zh")
                for jt in range(JT):
                    nc.vector.tensor_scalar(zh[:, jt, :], zt[:, jt, :],
                                            mrow[:, jt:jt + 1], rst[:, jt:jt + 1],
                                            OP.subtract, OP.mult)
                zhT = zsm.tile([128, JT, 128], BF, tag="zhT")
                for g in range(2):  # transpose 8 tiles, copy in 2 batches
                    pt = ztp.tile([128, 4, 128], BF, tag="t")
                    for q in range(4):
                        jt = g * 4 + q
                        nc.tensor.transpose(pt[:, q, :], zh[:, jt, :], ident[:])
                    nc.scalar.copy(out=zhT[:, g * 4:(g + 1) * 4, :], in_=pt[:])
                zb = zbp.tile([NB * H, S], F32, tag="zb")
                for jc in range(2):
                    nc.tensor.matmul(zb[:, bass.ts(jc, 512)], wz_sb[:],
                                     zhT[:].rearrange("p jt j -> p (jt j)")[:, bass.ts(jc, 512)],
                                     start=True, stop=False)
                    nc.tensor.matmul(zb[:, bass.ts(jc, 512)], sel_sb[:],
                                     bsel[:, bass.ts(jc, 512)],
                                     start=False, stop=True)
                zbs = zsm.tile([NB * H, S], BF, tag="zbs")
                nc.scalar.copy(out=zbs, in_=zb[:])
                nc.sync.dma_start(out=zbeta_dr[:, i, :], in_=zbs)

        # =========== block loop ===========
        wpool = ctx.enter_context(tc.tile_pool(name="wpool", bufs=2))
        wop = ctx.enter_context(tc.tile_pool(name="wop", bufs=1))
        blk = ctx.enter_context(tc.tile_pool(name="blk", bufs=1))
        kvg = ctx.enter_context(tc.tile_pool(name="kvg", bufs=1))
        att = ctx.enter_context(tc.tile_pool(name="att", bufs=3))
        dramc = ctx.enter_context(tc.tile_pool(name="dramc", bufs=2, space="DRAM"))

        for b in range(NB):
            # ---- ada_ln(a) shared stats ----
            with tc.tile_pool(name="lnp", bufs=1) as lnp, \
                 tc.tile_pool(name="ps_ln", bufs=2, space="PSUM") as pln:
                st3 = lnp.tile([R, 3, 6], F32)
                for g_ in range(3):
                    nc.vector.bn_stats(out=st3[:, g_, :],
                                       in_=a_sb[:, bass.ts(g_, 256)])
                mv = lnp.tile([R, 2], F32)
                nc.vector.bn_aggr(out=mv, in_=st3)
                rstd = lnp.tile([R, 1], F32)
                nc.scalar.activation(out=rstd, in_=mv[:, 1:2], func=ACTF.Sqrt,
                                     bias=biases[:, 0:1], scale=1.0)
                nc.vector.reciprocal(out=rstd, in_=rstd)
                xhat = lnp.tile([R, CA], BF)
                nc.vector.tensor_scalar(xhat[:], a_sb[:], mv[:, 0:1], rstd[:, 0:1],
                                        OP.subtract, OP.mult)
                ah = blk.tile([R, CA], BF, tag="ah")
                nc.vector.tensor_tensor(ah[:], xhat[:], sgsb[:, b, 0:CA], OP.mult)
                nc.vector.tensor_tensor(ah[:], ah[:], sgsb[:, b, CA:2 * CA], OP.add)
                th = blk.tile([R, CA], BF, tag="th")
                nc.vector.tensor_tensor(th[:], xhat[:], sgsb[:, b, 2 * CA:3 * CA], OP.mult)
                nc.vector.tensor_tensor(th[:], th[:], sgsb[:, b, 3 * CA:4 * CA], OP.add)
                ahT = blk.tile([128, KT_A, 128], BF, tag="ahT")
                for kt in range(KT_A):
                    pt = pln.tile([128, 128], BF, tag="t")
                    nc.tensor.transpose(pt[:], ah[:, bass.ts(kt, 128)], ident[:])
                    nc.scalar.copy(out=ahT[:, kt, :], in_=pt[:])
                thT = blk.tile([128, KT_A, 128], BF, tag="thT")
                for kt in range(KT_A):
                    pt = pln.tile([128, 128], BF, tag="t")
                    nc.tensor.transpose(pt[:], th[:, bass.ts(kt, 128)], ident[:])
                    nc.scalar.copy(out=thT[:, kt, :], in_=pt[:])

            # ---- kv local + gather ----
            KVB = KV_K + 2 * (KV_TOT - KV_K)
            kv_inb = dramc.tile([KVB], mybir.dt.uint8, tag="kvin")
            kv_outb = dramc.tile([NCORE * KVB], mybir.dt.uint8, tag="kvout",
                                 addr_space="Shared")
            with tc.tile_pool(name="ps_kv", bufs=3, space="PSUM") as pkv:
                wkv_sb = wpool.tile([128, KT_A, 2 * CA], BF, tag="w1536")
                nc.sync.dma_start(
                    out=wkv_sb,
                    in_=wkv_in[:].rearrange("(kt p) b m -> p kt b m", p=128)[:, :, b, :])
                kv_sb = kvg.tile([R, 2 * CA], BF, tag="kv")
                for m in range(3):  # 1536/512
                    ps = pkv.tile([R, 512], F32, tag="ps")
                    for kt in range(KT_A):
                        nc.tensor.matmul(ps[:], ahT[:, kt, :],
                                         wkv_sb[:, kt, bass.ts(m, 512)],
                                         start=(kt == 0), stop=(kt == KT_A - 1))
                    nc.scalar.copy(out=kv_sb[:, bass.ts(m, 512)], in_=ps[:])
                # kT head-tiles
                kT_loc = kvg.tile([D, H, 128], mybir.dt.float8e4, tag="kT")
                for h in range(H):
                    pt = pkv.tile([128, 128], BF, tag="t")
                    nc.tensor.transpose(pt[:D, :], kv_sb[:, h * D:(h + 1) * D], ident[:])
                    with nc.allow_low_precision(reason="k wire format fp8"):
                        nc.vector.tensor_copy(kT_loc[:, h, :], pt[:D, :])
                nc.sync.dma_start(
                    out=kv_inb[0:KV_K].rearrange("(d x) -> d x", d=D)
                        .bitcast(mybir.dt.float8e4),
                    in_=kT_loc[:].rearrange("d h j -> d (h j)"))
                nc.sync.dma_start(
                    out=kv_inb[KV_K:].rearrange("(j c) -> j c", j=R).bitcast(BF),
                    in_=kv_sb[:, CA:])
            nc.gpsimd.collective_compute(
                "AllGather", OP.bypass,
                replica_groups=[list(range(NCORE))],
                ins=[kv_inb[:].opt()], outs=[kv_outb[:].opt()])

            # ---- qT, gT (transposed head layout) ----
            with tc.tile_pool(name="ps_qg", bufs=3, space="PSUM") as pqg:
                wqg_sb = wpool.tile([128, KT_A, 2 * CA], BF, tag="w1536")
                nc.sync.dma_start(
                    out=wqg_sb,
                    in_=wqg_in[:].rearrange("(kt p) b m -> p kt b m", p=128)[:, :, b, :])
                bq_sb = blk.tile([D, H], F32, tag="bq")
                nc.sync.dma_start(
                    out=bq_sb, in_=bq_in[b].rearrange("(h d) -> d h", d=D))
                qT = blk.tile([D, H, 128], BF, tag="qT")
                gT = blk.tile([D, H, 128], BF, tag="gT")
                for h in range(H):
                    ps = pqg.tile([D, 128], F32, tag="ps")
                    for kt in range(KT_A):
                        nc.tensor.matmul(ps[:], wqg_sb[:, kt, h * D:(h + 1) * D],
                                         ahT[:, kt, :],
                                         start=(kt == 0), stop=(kt == KT_A - 1))
                    nc.scalar.activation(out=qT[:, h, :], in_=ps[:],
                                         func=ACTF.Identity,
                                         bias=bq_sb[:, h:h + 1], scale=1.0)
                    ps2 = pqg.tile([D, 128], F32, tag="ps")
                    for kt in range(KT_A):
                        nc.tensor.matmul(ps2[:], wqg_sb[:, kt, CA + h * D:CA + (h + 1) * D],
                                         ahT[:, kt, :],
                                         start=(kt == 0), stop=(kt == KT_A - 1))
                    nc.scalar.activation(out=gT[:, h, :], in_=ps2[:],
                                         func=ACTF.Sigmoid, bias=0.0, scale=1.0)

            # ---- transition: hiddenT = silu(th@wsw)^T * (th@wg2)^T ----
            hidT = blk.tile([128, KT_H, 128], BF, tag="hidT")
            with tc.tile_pool(name="ps_h", bufs=3, space="PSUM") as psh:
                wsw_sb = wpool.tile([128, KT_A, NHID], BF, tag="w1536")
                nc.sync.dma_start(
                    out=wsw_sb,
                    in_=wsw_in[:].rearrange("(kt p) b m -> p kt b m", p=128)[:, :, b, :])
                swT = blk.tile([128, KT_H, 128], BF, tag="swT")
                for mt in range(KT_H):
                    ps = psh.tile([128, 128], F32, tag="ps")
                    for kt in range(KT_A):
                        nc.tensor.matmul(ps[:], wsw_sb[:, kt, bass.ts(mt, 128)],
                                         thT[:, kt, :],
                                         start=(kt == 0), stop=(kt == KT_A - 1))
                    nc.scalar.activation(out=swT[:, mt, :], in_=ps[:],
                                         func=ACTF.Silu, bias=0.0, scale=1.0)
                wg2_sb = wpool.tile([128, KT_A, NHID], BF, tag="w1536")
                nc.sync.dma_start(
                    out=wg2_sb,
                    in_=wg2_in[:].rearrange("(kt p) b m -> p kt b m", p=128)[:, :, b, :])
                for mt in range(KT_H):
                    ps = psh.tile([128, 128], F32, tag="ps")
                    for kt in range(KT_A):
                        nc.tensor.matmul(ps[:], wg2_sb[:, kt, bass.ts(mt, 128)],
                                         thT[:, kt, :],
                                         start=(kt == 0), stop=(kt == KT_A - 1))
                    g2 = att.tile([128, 128], BF, tag="g2")
                    nc.scalar.copy(out=g2, in_=ps[:])
                    nc.vector.tensor_tensor(hidT[:, mt, :], swT[:, mt, :], g2[:],
                                            OP.mult)

            # ---- attention ----
            kv_outr = kv_outb[:].rearrange("(r x) -> r x", r=NCORE)
            kT_src = kv_outr[:, 0:KV_K].bitcast(mybir.dt.float8e4).rearrange(
                "r (d h j) -> d h r j", d=D, h=H)
            v_full = kvg.tile([128, NCORE, CA], BF, tag="vf")
            v_src = kv_outr[:, KV_K:].bitcast(BF) \
                .rearrange("r (j c) -> j r c", j=R)
            nc.sync.dma_start(out=v_full, in_=v_src[:])
            go_T = blk.tile([D, H, 128], BF, tag="goT")
            sums = blk.tile([R, H], F32, tag="sums")
            with tc.tile_pool(name="ps_s", bufs=2, space="PSUM") as pss, \
                 tc.tile_pool(name="ps_t", bufs=2, space="PSUM") as pst, \
                 tc.tile_pool(name="ps_o", bufs=2, space="PSUM") as pso:
                for h in range(H):
                    kT_f8 = att.tile([D, NCORE, 128], mybir.dt.float8e4, tag="kTf8")
                    nc.sync.dma_start(out=kT_f8, in_=kT_src[:, h, :, :])
                    kT_h = att.tile([D, NCORE, 128], BF, tag="kTh")
                    nc.vector.tensor_copy(kT_h[:], kT_f8[:])
                    zb_t = att.tile([R, S], BF, tag="zbt")
                    nc.sync.dma_start(out=zb_t, in_=zbeta_dr[b * H + h, :, :])
                    ps_s = pss.tile([R, S], F32, tag="s")
                    for jc in range(2):
                        nc.tensor.matmul(ps_s[:, bass.ts(jc, 512)], qT[:, h, :],
                                         kT_h[:, jc * 4:(jc + 1) * 4, :],
                                         start=True, stop=False)
                        nc.tensor.matmul(ps_s[:, bass.ts(jc, 512)], ident[:],
                                         zb_t[:, bass.ts(jc, 512)],
                                         start=False, stop=True,
                                         skip_group_check=True)
                    attn = att.tile([R, S], BF, tag="attn")
                    bh_ = 1 + b * H + h
                    nc.scalar.activation(out=attn, in_=ps_s[:], func=ACTF.Exp,
                                         bias=biases[:, bh_:bh_ + 1], scale=1.0,
                                         accum_out=sums[:, h:h + 1])
                    rec = att.tile([R, 1], F32, tag="rec")
                    nc.vector.reciprocal(out=rec, in_=sums[:, h:h + 1])
                    nc.vector.tensor_scalar(attn[:], attn[:], rec[:, 0:1], None,
                                            OP.mult)
                    attnT = att.tile([128, JT, 128], BF, tag="attnT")
                    for g in range(2):
                        pt = pst.tile([128, 4, 128], BF, tag="t")
                        for q in range(4):
                            jt = g * 4 + q
                            nc.tensor.transpose(pt[:, q, :],
                                                attn[:, bass.ts(jt, 128)], ident[:])
                        nc.vector.tensor_copy(attnT[:, g * 4:(g + 1) * 4, :], pt[:])
                    ps_o = pso.tile([128, 128], F32, tag="o")
                    for jt in range(JT):
                        nc.tensor.matmul(ps_o[:D, :], v_full[:, jt, h * D:(h + 1) * D],
                                         attnT[:, jt, :],
                                         start=(jt == 0), stop=(jt == JT - 1))
                    nc.vector.tensor_tensor(go_T[:, h, :], ps_o[:D, :],
                                            gT[:, h, :], OP.mult)

            # ---- att_out = (g*o) @ wo ; b_attn = gate_attn * att_out ----
            b_attn = blk.tile([R, CA], F32, tag="batt")
            with tc.tile_pool(name="ps_wo", bufs=2, space="PSUM") as pwo:
                wo_sb = wop.tile([D, H, CA], BF, tag="wo")
                nc.sync.dma_start(
                    out=wo_sb,
                    in_=wo_in[:].rearrange("(h d) b m -> d h b m", d=D)[:, :, b, :])
                for m in range(2):
                    n0, n1 = (0, 512) if m == 0 else (512, 768)
                    ps = pwo.tile([R, 512], F32, tag="ps")
                    for h in range(H):
                        nc.tensor.matmul(ps[:, 0:n1 - n0], go_T[:, h, :],
                                         wo_sb[:, h, n0:n1],
                                         start=(h == 0), stop=(h == H - 1))
                    nc.vector.tensor_tensor(b_attn[:, n0:n1], ps[:, 0:n1 - n0],
                                            gts[:, b, n0:n1], OP.mult)

            # ---- tr = gate_tr * (hidden @ w_out); a = b_attn + tr ----
            with tc.tile_pool(name="ps_tr", bufs=2, space="PSUM") as ptr:
                wout_sb = wop.tile([128, KT_H, CA], BF, tag="wout")
                nc.sync.dma_start(
                    out=wout_sb,
                    in_=wout_in[:].rearrange("(kt p) b m -> p kt b m", p=128)[:, :, b, :])
                for m in range(2):
                    n0, n1 = (0, 512) if m == 0 else (512, 768)
                    ps = ptr.tile([R, 512], F32, tag="ps")
                    for kt in range(KT_H):
                        nc.tensor.matmul(ps[:, 0:n1 - n0], hidT[:, kt, :],
                                         wout_sb[:, kt, n0:n1],
                                         start=(kt == 0), stop=(kt == KT_H - 1))
                    tr = att.tile([R, 512], F32, tag="tr")
                    nc.vector.tensor_tensor(tr[:, 0:n1 - n0], ps[:, 0:n1 - n0],
                                            gts[:, b, CA + n0:CA + n1], OP.mult)
                    nc.vector.tensor_tensor(a_sb[:, n0:n1], b_attn[:, n0:n1],
                                            tr[:, 0:n1 - n0], OP.add)

        nc.sync.dma_start(out=a_out[:], in_=a_sb[:])

    nc.finalize()
    return nc


def _prep_inputs(a, s, z, beta, ln_s_w_attn, wg_attn, wb_attn, wq, bq, wk, wv,
                 ln_z_w, ln_z_b, wpb, wgate, wo, wsg_attn, bsg_attn,
                 ln_s_w_tr, wg_tr, wb_tr, w_swish, w_gate2, wsg_tr, bsg_tr, w_out):
    bf = ml_dtypes.bfloat16
    f32 = np.float32
    scale = 1.0 / np.sqrt(np.float32(D))

    # folded weights (shared across cores)
    wz = np.concatenate([ln_z_w[i][:, None] * wpb[i] for i in range(NB)],
                        axis=1).astype(bf)                       # [CZ, NB*H]
    bias_pb = np.concatenate([ln_z_b[i] @ wpb[i] for i in range(NB)])  # [NB*H]
    sel = np.tile(np.eye(H, dtype=np.float32), (1, NB)).astype(bf)  # [H, NB*H]
    wsn = np.stack([np.concatenate(
        [ln_s_w_attn[i][:, None] * wg_attn[i], ln_s_w_attn[i][:, None] * wb_attn[i],
         ln_s_w_tr[i][:, None] * wg_tr[i], ln_s_w_tr[i][:, None] * wb_tr[i]],
        axis=1) for i in range(NB)], axis=1).astype(bf)          # [CS, NB, 4CA]
    wsr = np.stack([np.concatenate(
        [np.concatenate([wsg_attn[i], bsg_attn[i][None, :]], 0),
         np.concatenate([wsg_tr[i], bsg_tr[i][None, :]], 0)], axis=1)
        for i in range(NB)], axis=1).astype(bf)                  # [CS+1, NB, 2CA]
    wkv = np.stack([np.concatenate([wk[i], wv[i]], 1) for i in range(NB)],
                   axis=1).astype(bf)                            # [CA, NB, 2CA]
    wqg = np.stack([np.concatenate([wq[i] * scale, wgate[i]], 1)
                    for i in range(NB)], axis=1).astype(bf)
    bqe = (bq * scale).astype(f32)                               # [NB, CA]
    wsw = np.stack([w_swish[i] for i in range(NB)], axis=1).astype(bf)
    wg2 = np.stack([w_gate2[i] for i in range(NB)], axis=1).astype(bf)
    wob = np.stack([wo[i] for i in range(NB)], axis=1).astype(bf)
    wout = np.stack([w_out[i] for i in range(NB)], axis=1).astype(bf)

    shared = dict(wz_in=np.ascontiguousarray(wz),
                  sel_in=np.ascontiguousarray(sel),
                  wsn_in=np.ascontiguousarray(wsn),
                  wsr_in=np.ascontiguousarray(wsr),
                  wkv_in=np.ascontiguousarray(wkv),
                  wqg_in=np.ascontiguousarray(wqg),
                  bq_in=np.ascontiguousarray(bqe),
                  wsw_in=np.ascontiguousarray(wsw),
                  wg2_in=np.ascontiguousarray(wg2),
                  wo_in=np.ascontiguousarray(wob),
                  wout_in=np.ascontiguousarray(wout))

    a2 = a.reshape(S, CA).astype(f32)
    s2 = s.reshape(S, CS).astype(f32)
    z2 = z.reshape(S, S, CZ).astype(bf)
    betaT = np.ascontiguousarray(
        beta.reshape(S, S, H).transpose(2, 0, 1)).astype(bf)     # [H, S, S]

    in_maps = []
    for c in range(NCORE):
        rows = slice(c * R, (c + 1) * R)
        m = dict(shared)
        m["a_in"] = np.ascontiguousarray(a2[rows])
        m["s_in"] = np.ascontiguousarray(s2[rows])
        m["z_in"] = np.ascontiguousarray(
            z2[rows].reshape(R, JT, 128, CZ).transpose(0, 2, 1, 3))
        m["betaT_in"] = np.ascontiguousarray(betaT[:, rows, :])
        in_maps.append(m)
    return in_maps, [float(x) for x in bias_pb]


_CACHE = {}


def kernel(**inputs):
    inputs = {k: np.asarray(v) for k, v in inputs.items()}
    in_maps, bias_pb = _prep_inputs(**inputs)
    key = tuple(bias_pb)
    if key not in _CACHE:
        _CACHE.clear()
        _CACHE[key] = build_program(bias_pb)
    nc = _CACHE[key]
    res = run_bass_kernel_spmd(nc, in_maps, core_ids=list(range(NCORE)),
                               trace=False)
    out = np.concatenate([res.results[c]["a_out"] for c in range(NCORE)], axis=0)
    return out.reshape(1, S, CA).astype(np.float32)


if __name__ == "__main__":
    import reference
    ins = {k: np.asarray(v) for k, v in reference.setup_inputs().items()}
    exp = np.asarray(reference.reference(**ins))
    act = kernel(**ins)
    err = np.abs(act - exp).max() / (np.abs(exp).max() + 1e-9)
    print("rel err:", err)



# revision 50
# speedup vs baseline: 6.9622x; 1.0095x over previous
"""DiffusionTransformer (AF3-style) Trainium2 kernel, 8-core SPMD.

Sharding: sequence-parallel over rows (queries). Each core owns R=128 rows of
a / z / beta. Per block, k/v are computed on local rows and AllGathered
(bf16, ~384KB per rank). zbeta (pair bias LN(z)@wpb + beta, all 4 blocks) is
precomputed once into per-core DRAM in [bh, i, j] layout.
"""
import numpy as np
import ml_dtypes
from contextlib import ExitStack

import concourse.bass as bass
import concourse.mybir as mybir
import concourse.tile as tile
from concourse import bacc
from concourse.bass_utils import run_bass_kernel_spmd
from concourse.masks import make_identity

NB, H, S, CA, CS, CZ = 4, 16, 1024, 768, 384, 128
D = CA // H            # 48
NCORE = 8
R = S // NCORE         # 128 rows per core
NHID = 2 * CA          # 1536
EPS = 1e-5
BF = mybir.dt.bfloat16
F32 = mybir.dt.float32
AX = mybir.AxisListType
OP = mybir.AluOpType
ACTF = mybir.ActivationFunctionType
JT = S // 128          # 8 j-tiles
KT_A = CA // 128       # 6 k-tiles over c_a
KT_S = CS // 128       # 3 k-tiles over c_s
KT_H = NHID // 128     # 12 k-tiles over n_hidden
KV_K = D * H * R       # 98304 flat elems of kT part
KV_TOT = KV_K + R * CA  # + v part


def _headsplit_ranges():
    """Split [0,768) hd-range at both 128-tile and 48-head boundaries.

    Returns list of (kt, p0, p1, h, d0) with kt*128+p0 == h*48+d0.
    """
    out = []
    bounds = sorted(set([x * 128 for x in range(KT_A + 1)] +
                        [h * D for h in range(H + 1)]))
    for lo, hi in zip(bounds[:-1], bounds[1:]):
        kt, p0 = lo // 128, lo % 128
        h, d0 = lo // D, lo % D
        out.append((kt, p0, hi - lo, h, d0))
    return out


HS = _headsplit_ranges()


def build_program(bias_pb):
    nc = bacc.Bacc("TRN2", target_bir_lowering=False, debug=False,
                   num_devices=NCORE)

    # ---------------- I/O ----------------
    a_in = nc.dram_tensor("a_in", [R, CA], F32, kind="ExternalInput")
    s_in = nc.dram_tensor("s_in", [R, CS], F32, kind="ExternalInput")
    z_in = nc.dram_tensor("z_in", [R, 128, JT, CZ], BF, kind="ExternalInput")
    betaT_in = nc.dram_tensor("betaT_in", [H, R, S], BF, kind="ExternalInput")
    wz_in = nc.dram_tensor("wz_in", [CZ, NB * H], BF, kind="ExternalInput")
    sel_in = nc.dram_tensor("sel_in", [H, NB * H], BF, kind="ExternalInput")
    wsn_in = nc.dram_tensor("wsn_in", [CS, NB, 4 * CA], BF, kind="ExternalInput")
    wsr_in = nc.dram_tensor("wsr_in", [CS + 1, NB, 2 * CA], BF, kind="ExternalInput")
    wkv_in = nc.dram_tensor("wkv_in", [CA, NB, 2 * CA], BF, kind="ExternalInput")
    wqg_in = nc.dram_tensor("wqg_in", [CA, NB, 2 * CA], BF, kind="ExternalInput")
    bq_in = nc.dram_tensor("bq_in", [NB, CA], F32, kind="ExternalInput")
    wsw_in = nc.dram_tensor("wsw_in", [CA, NB, NHID], BF, kind="ExternalInput")
    wg2_in = nc.dram_tensor("wg2_in", [CA, NB, NHID], BF, kind="ExternalInput")
    wo_in = nc.dram_tensor("wo_in", [CA, NB, CA], BF, kind="ExternalInput")
    wout_in = nc.dram_tensor("wout_in", [NHID, NB, CA], BF, kind="ExternalInput")
    a_out = nc.dram_tensor("a_out", [R, CA], F32, kind="ExternalOutput")

    with tile.TileContext(nc) as tc, ExitStack() as ctx:
        const = ctx.enter_context(tc.tile_pool(name="const", bufs=1))
        ident = const.tile([128, 128], BF)
        make_identity(nc, ident)
        wz_sb = const.tile([CZ, NB * H], BF)
        nc.sync.dma_start(out=wz_sb, in_=wz_in[:])
        sel_sb = const.tile([H, NB * H], BF)
        nc.sync.dma_start(out=sel_sb, in_=sel_in[:])
        biases = const.tile([128, 1 + NB * H], F32)
        nc.vector.memset(biases[:, 0:1], EPS)
        for _bh in range(NB * H):
            nc.vector.memset(biases[:, 1 + _bh:2 + _bh], float(bias_pb[_bh]))

        pers = ctx.enter_context(tc.tile_pool(name="pers", bufs=1))
        a_sb = pers.tile([R, CA], F32)
        nc.sync.dma_start(out=a_sb, in_=a_in[:])

        # internal DRAM scratch
        dram = ctx.enter_context(tc.tile_pool(name="dram", bufs=1, space="DRAM"))
        zbeta_dr = dram.tile([NB * H, R, S], BF)

        # =========== s preprocessing (once) ===========
        with tc.tile_pool(name="sprep", bufs=1) as sp:
            s_sb = sp.tile([R, CS], F32)
            nc.sync.dma_start(out=s_sb, in_=s_in[:])
            stats = sp.tile([R, 1, 6], F32)
            mv = sp.tile([R, 2], F32)
            nc.vector.bn_stats(out=stats[:, 0, :], in_=s_sb[:])
            nc.vector.bn_aggr(out=mv, in_=stats)
            rstd = sp.tile([R, 1], F32)
            nc.scalar.activation(out=rstd, in_=mv[:, 1:2], func=ACTF.Sqrt,
                                 bias=biases[:, 0:1], scale=1.0)
            nc.vector.reciprocal(out=rstd, in_=rstd)
            s_n = sp.tile([R, CS], BF)
            nc.vector.tensor_scalar(s_n[:], s_sb[:], mv[:, 0:1], rstd[:, 0:1],
                                    OP.subtract, OP.mult)
            s_b16 = sp.tile([R, CS], BF)
            nc.vector.tensor_copy(s_b16[:], s_sb[:])

            # transposed copies (persistent for all blocks)
            s_nT = pers.tile([128, KT_S, 128], BF)
            sT = pers.tile([128, KT_S, 128], BF)
            ones_row = pers.tile([1, 128], BF)
            nc.vector.memset(ones_row, 1.0)
            with tc.tile_pool(name="tp_ps", bufs=2, space="PSUM") as tps:
                for kt in range(KT_S):
                    pt = tps.tile([128, 128], BF, tag="t")
                    nc.tensor.transpose(pt[:], s_n[:, bass.ts(kt, 128)], ident[:])
                    nc.scalar.copy(out=s_nT[:, kt, :], in_=pt[:])
                    pt2 = tps.tile([128, 128], BF, tag="t")
                    nc.tensor.transpose(pt2[:], s_b16[:, bass.ts(kt, 128)], ident[:])
                    nc.scalar.copy(out=sT[:, kt, :], in_=pt2[:])

            # per-block s-derived tensors: sg/sb for attn+tr, gates attn/tr
            sgsb = pers.tile([R, NB, 4 * CA], BF)   # wg_a|wb_a|wg_t|wb_t
            gts = pers.tile([R, NB, 2 * CA], BF)    # gate_attn|gate_tr
            with tc.tile_pool(name="sw", bufs=2) as swp, \
                 tc.tile_pool(name="sps", bufs=3, space="PSUM") as sps:
                for b in range(NB):
                    wsn_sb = swp.tile([128, KT_S, 4 * CA], BF, tag="wsn")
                    nc.sync.dma_start(
                        out=wsn_sb,
                        in_=wsn_in[:].rearrange("(kt p) b m -> p kt b m", p=128)[:, :, b, :])
                    for m in range(6):  # 3072 / 512
                        ps = sps.tile([R, 512], F32, tag="ps")
                        for kt in range(KT_S):
                            nc.tensor.matmul(ps[:], s_nT[:, kt, :],
                                             wsn_sb[:, kt, bass.ts(m, 512)],
                                             start=(kt == 0), stop=(kt == KT_S - 1))
                        # cols [m*512,(m+1)*512) of [wg_a(768)|wb_a|wg_t|wb_t]
                        for lo_, hi_ in [(m * 512, m * 512 + 256), (m * 512 + 256, (m + 1) * 512)]:
                            mat = lo_ // CA  # 0..3
                            f = ACTF.Sigmoid if mat in (0, 2) else ACTF.Copy
                            nc.scalar.activation(
                                out=sgsb[:, b, lo_:hi_], in_=ps[:, lo_ - m * 512:hi_ - m * 512],
                                func=f, bias=0.0 if f == ACTF.Copy else 0.0, scale=1.0)
                    wsr_sb = swp.tile([128, KT_S, 2 * CA], BF, tag="wsr")
                    nc.sync.dma_start(
                        out=wsr_sb,
                        in_=wsr_in[:CS].rearrange("(kt p) b m -> p kt b m", p=128)[:, :, b, :])
                    wsr_last = swp.tile([1, 2 * CA], BF, tag="wsrl")
                    nc.sync.dma_start(out=wsr_last, in_=wsr_in[CS:CS + 1, b, :])
                    for m in range(3):  # 1536 / 512
                        ps = sps.tile([R, 512], F32, tag="ps")
                        for kt in range(KT_S):
                            nc.tensor.matmul(ps[:], sT[:, kt, :],
                                             wsr_sb[:, kt, bass.ts(m, 512)],
                                             start=(kt == 0), stop=False)
                        nc.tensor.matmul(ps[:], ones_row[:],
                                         wsr_last[:, bass.ts(m, 512)],
                                         start=False, stop=True)
                        nc.scalar.activation(out=gts[:, b, bass.ts(m, 512)],
                                             in_=ps[:], func=ACTF.Sigmoid,
                                             bias=0.0, scale=1.0)

        # =========== z preprocessing (once) ===========
        with tc.tile_pool(name="zslab", bufs=4) as zsl, \
             tc.tile_pool(name="zsm", bufs=4) as zsm, \
             tc.tile_pool(name="ztp", bufs=3, space="PSUM") as ztp, \
             tc.tile_pool(name="zbp", bufs=2, space="PSUM") as zbp:
            for i in range(R):
                zt = zsl.tile([128, JT, CZ], BF, tag="z")
                nc.sync.dma_start(out=zt, in_=z_in[i])
                bsel = zsl.tile([H, S], BF, tag="bsel")
                nc.sync.dma_start(out=bsel, in_=betaT_in[:, i, :])
                st8 = zsm.tile([128, JT, 6], F32, tag="st")
                for jt in range(JT):
                    nc.vector.bn_stats(out=st8[:, jt, :], in_=zt[:, jt, :])
                # pooled even/odd moments, vectorized over all 8 j-tiles:
                # mean = (m_e+m_o)/2; var = (64v_e+64v_o)/128 + ((m_e-m_o)/2)^2
                mrow = zsm.tile([128, JT], F32, tag="mrow")
                nc.gpsimd.tensor_tensor(mrow[:], st8[:, :, 1], st8[:, :, 4], OP.add)
                nc.vector.tensor_scalar(mrow[:], mrow[:], 0.5, None, OP.mult)
                dm = zsm.tile([128, JT], F32, tag="dm")
                nc.gpsimd.tensor_tensor(dm[:], st8[:, :, 1], st8[:, :, 4],
                                        OP.subtract)
                nc.gpsimd.tensor_tensor(dm[:], dm[:], dm[:], OP.mult)
                nc.vector.tensor_scalar(dm[:], dm[:], 0.25, None, OP.mult)
                var = zsm.tile([128, JT], F32, tag="var")
                nc.gpsimd.tensor_tensor(var[:], st8[:, :, 2], st8[:, :, 5], OP.add)
                nc.vector.tensor_scalar(var[:], var[:], 1.0 / CZ, None, OP.mult)
                nc.vector.tensor_tensor(var[:], var[:], dm[:], OP.add)
                rst = zsm.tile([128, JT], F32, tag="rst")
                nc.scalar.activation(out=rst, in_=var[:], func=ACTF.Sqrt,
                                     bias=biases[:, 0:1], scale=1.0)
                nc.vector.reciprocal(out=rst, in_=rst)
                zh = zsm.tile([128, JT, CZ], BF, tag="zh")
                for jt in range(JT):
                    nc.vector.tensor_scalar(zh[:, jt, :], zt[:, jt, :],
                                            mrow[:, jt:jt + 1], rst[:, jt:jt + 1],
                                            OP.subtract, OP.mult)
                zhT = zsm.tile([128, JT, 128], BF, tag="zhT")
                for g in range(2):  # transpose 8 tiles, copy in 2 batches
                    pt = ztp.tile([128, 4, 128], BF, tag="t")
                    for q in range(4):
                        jt = g * 4 + q
                        nc.tensor.transpose(pt[:, q, :], zh[:, jt, :], ident[:])
                    nc.scalar.copy(out=zhT[:, g * 4:(g + 1) * 4, :], in_=pt[:])
                zb = zbp.tile([NB * H, S], F32, tag="zb")
                for jc in range(2):
                    nc.tensor.matmul(zb[:, bass.ts(jc, 512)], wz_sb[:],
                                     zhT[:].rearrange("p jt j -> p (jt j)")[:, bass.ts(jc, 512)],
                                     start=True, stop=False)
                    nc.tensor.matmul(zb[:, bass.ts(jc, 512)], sel_sb[:],
                                     bsel[:, bass.ts(jc, 512)],
                                     start=False, stop=True)
                zbs = zsm.tile([NB * H, S], BF, tag="zbs")
                nc.scalar.copy(out=zbs, in_=zb[:])
                nc.sync.dma_start(out=zbeta_dr[:, i, :], in_=zbs)

        # =========== block loop ===========
        wpool = ctx.enter_context(tc.tile_pool(name="wpool", bufs=2))
        wop = ctx.enter_context(tc.tile_pool(name="wop", bufs=1))
        blk = ctx.enter_context(tc.tile_pool(name="blk", bufs=1))
        kvg = ctx.enter_context(tc.tile_pool(name="kvg", bufs=1))
        att = ctx.enter_context(tc.tile_pool(name="att", bufs=3))
        dramc = ctx.enter_context(tc.tile_pool(name="dramc", bufs=2, space="DRAM"))

        for b in range(NB):
            # ---- ada_ln(a) shared stats ----
            with tc.tile_pool(name="lnp", bufs=1) as lnp, \
                 tc.tile_pool(name="ps_ln", bufs=2, space="PSUM") as pln:
                st3 = lnp.tile([R, 3, 6], F32)
                for g_ in range(3):
                    nc.vector.bn_stats(out=st3[:, g_, :],
                                       in_=a_sb[:, bass.ts(g_, 256)])
                mv = lnp.tile([R, 2], F32)
                nc.vector.bn_aggr(out=mv, in_=st3)
                rstd = lnp.tile([R, 1], F32)
                nc.scalar.activation(out=rstd, in_=mv[:, 1:2], func=ACTF.Sqrt,
                                     bias=biases[:, 0:1], scale=1.0)
                nc.vector.reciprocal(out=rstd, in_=rstd)
                xhat = lnp.tile([R, CA], BF)
                nc.vector.tensor_scalar(xhat[:], a_sb[:], mv[:, 0:1], rstd[:, 0:1],
                                        OP.subtract, OP.mult)
                ah = blk.tile([R, CA], BF, tag="ah")
                nc.vector.tensor_tensor(ah[:], xhat[:], sgsb[:, b, 0:CA], OP.mult)
                nc.vector.tensor_tensor(ah[:], ah[:], sgsb[:, b, CA:2 * CA], OP.add)
                th = blk.tile([R, CA], BF, tag="th")
                nc.vector.tensor_tensor(th[:], xhat[:], sgsb[:, b, 2 * CA:3 * CA], OP.mult)
                nc.vector.tensor_tensor(th[:], th[:], sgsb[:, b, 3 * CA:4 * CA], OP.add)
                ahT = blk.tile([128, KT_A, 128], BF, tag="ahT")
                for kt in range(KT_A):
                    pt = pln.tile([128, 128], BF, tag="t")
                    nc.tensor.transpose(pt[:], ah[:, bass.ts(kt, 128)], ident[:])
                    nc.scalar.copy(out=ahT[:, kt, :], in_=pt[:])
                thT = blk.tile([128, KT_A, 128], BF, tag="thT")
                for kt in range(KT_A):
                    pt = pln.tile([128, 128], BF, tag="t")
                    nc.tensor.transpose(pt[:], th[:, bass.ts(kt, 128)], ident[:])
                    nc.scalar.copy(out=thT[:, kt, :], in_=pt[:])

            # ---- kv local + gather ----
            KVB = KV_K + 2 * (KV_TOT - KV_K)
            kv_inb = dramc.tile([KVB], mybir.dt.uint8, tag="kvin")
            kv_outb = dramc.tile([NCORE * KVB], mybir.dt.uint8, tag="kvout",
                                 addr_space="Shared")
            with tc.tile_pool(name="ps_kv", bufs=3, space="PSUM") as pkv:
                wkv_sb = wpool.tile([128, KT_A, 2 * CA], BF, tag="w1536")
                nc.sync.dma_start(
                    out=wkv_sb,
                    in_=wkv_in[:].rearrange("(kt p) b m -> p kt b m", p=128)[:, :, b, :])
                kv_sb = kvg.tile([R, 2 * CA], BF, tag="kv")
                for m in range(3):  # 1536/512
                    ps = pkv.tile([R, 512], F32, tag="ps")
                    for kt in range(KT_A):
                        nc.tensor.matmul(ps[:], ahT[:, kt, :],
                                         wkv_sb[:, kt, bass.ts(m, 512)],
                                         start=(kt == 0), stop=(kt == KT_A - 1))
                    nc.scalar.copy(out=kv_sb[:, bass.ts(m, 512)], in_=ps[:])
                # kT head-tiles
                kT_loc = kvg.tile([D, H, 128], mybir.dt.float8e4, tag="kT")
                for h in range(H):
                    pt = pkv.tile([128, 128], BF, tag="t")
                    nc.tensor.transpose(pt[:D, :], kv_sb[:, h * D:(h + 1) * D], ident[:])
                    with nc.allow_low_precision(reason="k wire format fp8"):
                        nc.vector.tensor_copy(kT_loc[:, h, :], pt[:D, :])
                nc.sync.dma_start(
                    out=kv_inb[0:KV_K].rearrange("(d x) -> d x", d=D)
                        .bitcast(mybir.dt.float8e4),
                    in_=kT_loc[:].rearrange("d h j -> d (h j)"))
                nc.sync.dma_start(
                    out=kv_inb[KV_K:].rearrange("(j c) -> j c", j=R).bitcast(BF),
                    in_=kv_sb[:, CA:])
            nc.gpsimd.collective_compute(
                "AllGather", OP.bypass,
                replica_groups=[list(range(NCORE))],
                ins=[kv_inb[:].opt()], outs=[kv_outb[:].opt()])

            # ---- qT, gT (transposed head layout) ----
            with tc.tile_pool(name="ps_qg", bufs=3, space="PSUM") as pqg:
                wqg_sb = wpool.tile([128, KT_A, 2 * CA], BF, tag="w1536")
                nc.sync.dma_start(
                    out=wqg_sb,
                    in_=wqg_in[:].rearrange("(kt p) b m -> p kt b m", p=128)[:, :, b, :])
                bq_sb = blk.tile([D, H], F32, tag="bq")
                nc.sync.dma_start(
                    out=bq_sb, in_=bq_in[b].rearrange("(h d) -> d h", d=D))
                qT = blk.tile([D, H, 128], mybir.dt.float8e4, tag="qT")
                gT = blk.tile([D, H, 128], BF, tag="gT")
                for h in range(H):
                    ps = pqg.tile([D, 128], F32, tag="ps")
                    for kt in range(KT_A):
                        nc.tensor.matmul(ps[:], wqg_sb[:, kt, h * D:(h + 1) * D],
                                         ahT[:, kt, :],
                                         start=(kt == 0), stop=(kt == KT_A - 1))
                    with nc.allow_low_precision(reason="q fp8 for fp8 qk"):
                        nc.scalar.activation(out=qT[:, h, :], in_=ps[:],
                                             func=ACTF.Identity,
                                             bias=bq_sb[:, h:h + 1], scale=1.0)
                    ps2 = pqg.tile([D, 128], F32, tag="ps")
                    for kt in range(KT_A):
                        nc.tensor.matmul(ps2[:], wqg_sb[:, kt, CA + h * D:CA + (h + 1) * D],
                                         ahT[:, kt, :],
                                         start=(kt == 0), stop=(kt == KT_A - 1))
                    nc.scalar.activation(out=gT[:, h, :], in_=ps2[:],
                                         func=ACTF.Sigmoid, bias=0.0, scale=1.0)

            # ---- transition: hiddenT = silu(th@wsw)^T * (th@wg2)^T ----
            hidT = blk.tile([128, KT_H, 128], BF, tag="hidT")
            with tc.tile_pool(name="ps_h", bufs=3, space="PSUM") as psh:
                wsw_sb = wpool.tile([128, KT_A, NHID], BF, tag="w1536")
                nc.sync.dma_start(
                    out=wsw_sb,
                    in_=wsw_in[:].rearrange("(kt p) b m -> p kt b m", p=128)[:, :, b, :])
                swT = blk.tile([128, KT_H, 128], BF, tag="swT")
                for mt in range(KT_H):
                    ps = psh.tile([128, 128], F32, tag="ps")
                    for kt in range(KT_A):
                        nc.tensor.matmul(ps[:], wsw_sb[:, kt, bass.ts(mt, 128)],
                                         thT[:, kt, :],
                                         start=(kt == 0), stop=(kt == KT_A - 1))
                    nc.scalar.activation(out=swT[:, mt, :], in_=ps[:],
                                         func=ACTF.Silu, bias=0.0, scale=1.0)
                wg2_sb = wpool.tile([128, KT_A, NHID], BF, tag="w1536")
                nc.sync.dma_start(
                    out=wg2_sb,
                    in_=wg2_in[:].rearrange("(kt p) b m -> p kt b m", p=128)[:, :, b, :])
                for mt in range(KT_H):
                    ps = psh.tile([128, 128], F32, tag="ps")
                    for kt in range(KT_A):
                        nc.tensor.matmul(ps[:], wg2_sb[:, kt, bass.ts(mt, 128)],
                                         thT[:, kt, :],
                                         start=(kt == 0), stop=(kt == KT_A - 1))
                    g2 = att.tile([128, 128], BF, tag="g2")
                    nc.scalar.copy(out=g2, in_=ps[:])
                    nc.vector.tensor_tensor(hidT[:, mt, :], swT[:, mt, :], g2[:],
                                            OP.mult)

            # ---- attention ----
            kv_outr = kv_outb[:].rearrange("(r x) -> r x", r=NCORE)
            kT_src = kv_outr[:, 0:KV_K].bitcast(mybir.dt.float8e4).rearrange(
                "r (d h j) -> d h r j", d=D, h=H)
            v_full = kvg.tile([128, NCORE, CA], BF, tag="vf")
            v_src = kv_outr[:, KV_K:].bitcast(BF) \
                .rearrange("r (j c) -> j r c", j=R)
            nc.sync.dma_start(out=v_full, in_=v_src[:])
            go_T = blk.tile([D, H, 128], BF, tag="goT")
            sums = blk.tile([R, H], F32, tag="sums")
            with tc.tile_pool(name="ps_s", bufs=2, space="PSUM") as pss, \
                 tc.tile_pool(name="ps_t", bufs=2, space="PSUM") as pst, \
                 tc.tile_pool(name="ps_o", bufs=2, space="PSUM") as pso:
                for h in range(H):
                    kT_h = att.tile([D, NCORE, 128], mybir.dt.float8e4, tag="kTh")
                    nc.sync.dma_start(out=kT_h, in_=kT_src[:, h, :, :])
                    zb_t = att.tile([R, S], BF, tag="zbt")
                    nc.sync.dma_start(out=zb_t, in_=zbeta_dr[b * H + h, :, :])
                    ps_s = pss.tile([R, S], F32, tag="s")
                    for jc in range(2):
                        nc.tensor.matmul(ps_s[:, bass.ts(jc, 512)], qT[:, h, :],
                                         kT_h[:, jc * 4:(jc + 1) * 4, :],
                                         start=True, stop=False)
                        nc.tensor.matmul(ps_s[:, bass.ts(jc, 512)], ident[:],
                                         zb_t[:, bass.ts(jc, 512)],
                                         start=False, stop=True,
                                         skip_group_check=True)
                    attn = att.tile([R, S], BF, tag="attn")
                    bh_ = 1 + b * H + h
                    nc.scalar.activation(out=attn, in_=ps_s[:], func=ACTF.Exp,
                                         bias=biases[:, bh_:bh_ + 1], scale=1.0,
                                         accum_out=sums[:, h:h + 1])
                    rec = att.tile([R, 1], F32, tag="rec")
                    nc.vector.reciprocal(out=rec, in_=sums[:, h:h + 1])
                    nc.vector.tensor_scalar(attn[:], attn[:], rec[:, 0:1], None,
                                            OP.mult)
                    attnT = att.tile([128, JT, 128], BF, tag="attnT")
                    for g in range(2):
                        pt = pst.tile([128, 4, 128], BF, tag="t")
                        for q in range(4):
                            jt = g * 4 + q
                            nc.tensor.transpose(pt[:, q, :],
                                                attn[:, bass.ts(jt, 128)], ident[:])
                        nc.vector.tensor_copy(attnT[:, g * 4:(g + 1) * 4, :], pt[:])
                    ps_o = pso.tile([128, 128], F32, tag="o")
                    for jt in range(JT):
                        nc.tensor.matmul(ps_o[:D, :], v_full[:, jt, h * D:(h + 1) * D],
                                         attnT[:, jt, :],
                                         start=(jt == 0), stop=(jt == JT - 1))
                    nc.vector.tensor_tensor(go_T[:, h, :], ps_o[:D, :],
                                            gT[:, h, :], OP.mult)

            # ---- att_out = (g*o) @ wo ; b_attn = gate_attn * att_out ----
            b_attn = blk.tile([R, CA], F32, tag="batt")
            with tc.tile_pool(name="ps_wo", bufs=2, space="PSUM") as pwo:
                wo_sb = wop.tile([D, H, CA], BF, tag="wo")
                nc.sync.dma_start(
                    out=wo_sb,
                    in_=wo_in[:].rearrange("(h d) b m -> d h b m", d=D)[:, :, b, :])
                for m in range(2):
                    n0, n1 = (0, 512) if m == 0 else (512, 768)
                    ps = pwo.tile([R, 512], F32, tag="ps")
                    for h in range(H):
                        nc.tensor.matmul(ps[:, 0:n1 - n0], go_T[:, h, :],
                                         wo_sb[:, h, n0:n1],
                                         start=(h == 0), stop=(h == H - 1))
                    nc.vector.tensor_tensor(b_attn[:, n0:n1], ps[:, 0:n1 - n0],
                                            gts[:, b, n0:n1], OP.mult)

            # ---- tr = gate_tr * (hidden @ w_out); a = b_attn + tr ----
            with tc.tile_pool(name="ps_tr", bufs=2, space="PSUM") as ptr:
                wout_sb = wop.tile([128, KT_H, CA], BF, tag="wout")
                nc.sync.dma_start(
                    out=wout_sb,
                    in_=wout_in[:].rearrange("(kt p) b m -> p kt b m", p=128)[:, :, b, :])
                for m in range(2):
                    n0, n1 = (0, 512) if m == 0 else (512, 768)
                    ps = ptr.tile([R, 512], F32, tag="ps")
                    for kt in range(KT_H):
                        nc.tensor.matmul(ps[:, 0:n1 - n0], hidT[:, kt, :],
                                         wout_sb[:, kt, n0:n1],
                                         start=(kt == 0), stop=(kt == KT_H - 1))
                    tr = att.tile([R, 512], F32, tag="tr")
                    nc.vector.tensor_tensor(tr[:, 0:n1 - n0], ps[:, 0:n1 - n0],
                                            gts[:, b, CA + n0:CA + n1], OP.mult)
                    nc.vector.tensor_tensor(a_sb[:, n0:n1], b_attn[:, n0:n1],
                                            tr[:, 0:n1 - n0], OP.add)

        nc.sync.dma_start(out=a_out[:], in_=a_sb[:])

    nc.finalize()
    return nc


def _prep_inputs(a, s, z, beta, ln_s_w_attn, wg_attn, wb_attn, wq, bq, wk, wv,
                 ln_z_w, ln_z_b, wpb, wgate, wo, wsg_attn, bsg_attn,
                 ln_s_w_tr, wg_tr, wb_tr, w_swish, w_gate2, wsg_tr, bsg_tr, w_out):
    bf = ml_dtypes.bfloat16
    f32 = np.float32
    scale = 1.0 / np.sqrt(np.float32(D))

    # folded weights (shared across cores)
    wz = np.concatenate([ln_z_w[i][:, None] * wpb[i] for i in range(NB)],
                        axis=1).astype(bf)                       # [CZ, NB*H]
    bias_pb = np.concatenate([ln_z_b[i] @ wpb[i] for i in range(NB)])  # [NB*H]
    sel = np.tile(np.eye(H, dtype=np.float32), (1, NB)).astype(bf)  # [H, NB*H]
    wsn = np.stack([np.concatenate(
        [ln_s_w_attn[i][:, None] * wg_attn[i], ln_s_w_attn[i][:, None] * wb_attn[i],
         ln_s_w_tr[i][:, None] * wg_tr[i], ln_s_w_tr[i][:, None] * wb_tr[i]],
        axis=1) for i in range(NB)], axis=1).astype(bf)          # [CS, NB, 4CA]
    wsr = np.stack([np.concatenate(
        [np.concatenate([wsg_attn[i], bsg_attn[i][None, :]], 0),
         np.concatenate([wsg_tr[i], bsg_tr[i][None, :]], 0)], axis=1)
        for i in range(NB)], axis=1).astype(bf)                  # [CS+1, NB, 2CA]
    wkv = np.stack([np.concatenate([wk[i], wv[i]], 1) for i in range(NB)],
                   axis=1).astype(bf)                            # [CA, NB, 2CA]
    wqg = np.stack([np.concatenate([wq[i] * scale, wgate[i]], 1)
                    for i in range(NB)], axis=1).astype(bf)
    bqe = (bq * scale).astype(f32)                               # [NB, CA]
    wsw = np.stack([w_swish[i] for i in range(NB)], axis=1).astype(bf)
    wg2 = np.stack([w_gate2[i] for i in range(NB)], axis=1).astype(bf)
    wob = np.stack([wo[i] for i in range(NB)], axis=1).astype(bf)
    wout = np.stack([w_out[i] for i in range(NB)], axis=1).astype(bf)

    shared = dict(wz_in=np.ascontiguousarray(wz),
                  sel_in=np.ascontiguousarray(sel),
                  wsn_in=np.ascontiguousarray(wsn),
                  wsr_in=np.ascontiguousarray(wsr),
                  wkv_in=np.ascontiguousarray(wkv),
                  wqg_in=np.ascontiguousarray(wqg),
                  bq_in=np.ascontiguousarray(bqe),
                  wsw_in=np.ascontiguousarray(wsw),
                  wg2_in=np.ascontiguousarray(wg2),
                  wo_in=np.ascontiguousarray(wob),
                  wout_in=np.ascontiguousarray(wout))

    a2 = a.reshape(S, CA).astype(f32)
    s2 = s.reshape(S, CS).astype(f32)
    z2 = z.reshape(S, S, CZ).astype(bf)
    betaT = np.ascontiguousarray(
        beta.reshape(S, S, H).transpose(2, 0, 1)).astype(bf)     # [H, S, S]

    in_maps = []
    for c in range(NCORE):
        rows = slice(c * R, (c + 1) * R)
        m = dict(shared)
        m["a_in"] = np.ascontiguousarray(a2[rows])
        m["s_in"] = np.ascontiguousarray(s2[rows])
        m["z_in"] = np.ascontiguousarray(
            z2[rows].reshape(R, JT, 128, CZ).transpose(0, 2, 1, 3))
        m["betaT_in"] = np.ascontiguousarray(betaT[:, rows, :])
        in_maps.append(m)
    return in_maps, [float(x) for x in bias_pb]


_CACHE = {}


def kernel(**inputs):
    inputs = {k: np.asarray(v) for k, v in inputs.items()}
    in_maps, bias_pb = _prep_inputs(**inputs)
    key = tuple(bias_pb)
    if key not in _CACHE:
        _CACHE.clear()
        _CACHE[key] = build_program(bias_pb)
    nc = _CACHE[key]
    res = run_bass_kernel_spmd(nc, in_maps, core_ids=list(range(NCORE)),
                               trace=False)
    out = np.concatenate([res.results[c]["a_out"] for c in range(NCORE)], axis=0)
    return out.reshape(1, S, CA).astype(np.float32)


if __name__ == "__main__":
    import reference
    ins = {k: np.asarray(v) for k, v in reference.setup_inputs().items()}
    exp = np.asarray(reference.reference(**ins))
    act = kernel(**ins)
    err = np.abs(act - exp).max() / (np.abs(exp).max() + 1e-9)
    print("rel err:", err)

